# revision 1
# baseline (speedup 1.0000x reference)
"""GQA attention (SEQ=2048, DIM=4096, 32 Q heads / 8 KV heads, head_dim=128),
tensor-parallel over heads across 8 NeuronCores.

Each core owns 4 Q heads + 1 KV head: wq/wk/wv split column-wise, wo split
row-wise; each core produces a partial (2048, 4096) output that the host sums
(the all-reduce of row-parallel wo).

Per-core kernel (matmuls on the float32r PE path: full fp32 operand bytes,
tf32-like rounding, 1 cyc/row at free-dim 512 vs 4 cyc/row for plain fp32):
  A) QKV projections: stream xT (dim-major) blocks; Q^T/K^T/V^T accumulate in
     PSUM over the 4096 contraction; RoPE applied on PSUM eviction; V^T
     transposed back to V via PE transposes.
  B) Flash-style attention per (head, 512-query block): S^T = K^T_blk.T @ Q^T
     (keys on partitions), causal staircase mask added on diagonal blocks,
     exp on ACT (scale=1/sqrt(128) folded in), D = ones.T @ expS^T summed over
     key blocks on the PE, O^T = V_blk.T @ expS^T accumulated in PSUM,
     normalized by 1/D (PE broadcast of the reciprocal) on eviction.
  C) out = O^T.T @ wo accumulated over the 4 heads, streamed to DRAM.
"""

import numpy as np

import concourse.bacc as bacc
import concourse.tile as tile
from concourse import mybir
from concourse.bass_utils import run_bass_kernel_spmd

F32 = mybir.dt.float32
F32R = mybir.dt.float32r

DIM = 4096
SEQ = 2048
HEAD_DIM = 128
N_CORES = 8
QH = 4            # q heads per core
QS = QH * HEAD_DIM  # 512: wq column slice per core
NKT = DIM // 128    # 32 contraction tiles
NSB = SEQ // 512    # 4 sequence blocks
SCALE = 1.0 / float(np.sqrt(HEAD_DIM))
NEG = -1e9


def build_nc():
    nc = bacc.Bacc(trn_type="TRN2")

    xT = nc.declare_dram_parameter("xT", [DIM, SEQ], F32R, isOutput=False)
    wq = nc.declare_dram_parameter("wq", [DIM, QS], F32R, isOutput=False)
    wk = nc.declare_dram_parameter("wk", [DIM, HEAD_DIM], F32R, isOutput=False)
    wv = nc.declare_dram_parameter("wv", [DIM, HEAD_DIM], F32R, isOutput=False)
    wo = nc.declare_dram_parameter("wo", [QS, DIM], F32R, isOutput=False)
    cosT = nc.declare_dram_parameter("cosT", [HEAD_DIM, SEQ], F32, isOutput=False)
    sinTs = nc.declare_dram_parameter("sinTs", [HEAD_DIM, SEQ], F32, isOutput=False)
    stair = nc.declare_dram_parameter("stair", [128, 896], F32, isOutput=False)
    ident = nc.declare_dram_parameter("ident", [128, 128], F32R, isOutput=False)
    ones_col = nc.declare_dram_parameter("ones_col", [128, 1], F32R, isOutput=False)
    ones_row = nc.declare_dram_parameter("ones_row", [1, 128], F32R, isOutput=False)
    out = nc.declare_dram_parameter("out", [SEQ, DIM], F32, isOutput=True)

    with tile.TileContext(nc) as tc:
        with (
            tc.tile_pool(name="persist", bufs=1) as persist,
            tc.tile_pool(name="resid", bufs=1) as resid,
        ):
            # small constants
            stair_sb = persist.tile([128, 896], F32)
            nc.sync.dma_start(out=stair_sb, in_=stair[:, :])
            ident_sb = persist.tile([128, 128], F32R)
            nc.sync.dma_start(out=ident_sb, in_=ident[:, :])
            onesc_sb = persist.tile([128, 1], F32R)
            nc.sync.dma_start(out=onesc_sb, in_=ones_col[:, :])
            onesr_sb = persist.tile([1, 128], F32R)
            nc.sync.dma_start(out=onesr_sb, in_=ones_row[:, :])

            # resident activations
            qT = resid.tile([128, QH, SEQ], F32R)      # Q^T per head (d, seq)
            kT = resid.tile([128, SEQ], F32R)          # K^T (d, seq)
            vN = resid.tile([128, SEQ // 128, 128], F32R)  # V natural (keys, d)

            # ---------------- Phase A: projections + RoPE ----------------
            with (
                tc.tile_pool(name="wpool", bufs=1) as wpool,
                tc.tile_pool(name="xpool", bufs=2) as xpool,
                tc.tile_pool(name="cspool", bufs=2) as cspool,
                tc.tile_pool(name="ropetmp", bufs=2) as ropetmp,
                tc.tile_pool(name="vtb", bufs=2) as vtb,
                tc.tile_pool(name="psA", bufs=1, space="PSUM") as psA,
                tc.tile_pool(name="psVT", bufs=2, space="PSUM") as psVT,
            ):
                # resident weights: per-4kt chunk tiles so the first matmuls
                # only wait on their own 1 MiB DMA, not the whole weight load
                wq_r = wq.rearrange("(t p) m -> p t m", p=128)
                wk_r = wk.rearrange("(t p) m -> p t m", p=128)
                wv_r = wv.rearrange("(t p) m -> p t m", p=128)
                wq_cs, wk_cs, wv_cs = [], [], []
                for c in range(8):
                    wq_cs.append(wpool.tile([128, 4, QS], F32R, name=f"wqc{c}"))
                    wk_cs.append(wpool.tile([128, 4, HEAD_DIM], F32R, name=f"wkc{c}"))
                    wv_cs.append(wpool.tile([128, 4, HEAD_DIM], F32R, name=f"wvc{c}"))

                xT_r = xT.rearrange("(t p) s -> p t s", p=128)

                for sb in range(NSB):
                    ss = slice(sb * 512, (sb + 1) * 512)
                    # PSUM accumulators for this seq block
                    q_ps = [psA.tile([128, 512], F32, tag=f"qps{h}", name=f"qps{h}")
                            for h in range(QH)]
                    k_ps = psA.tile([128, 512], F32, tag="kps")
                    v_ps = psA.tile([128, 512], F32, tag="vps")

                    for g in range(8):  # super-tiles of 4 k-tiles (1 MiB DMAs)
                        if sb == 0:
                            # interleave weight chunks with the x stream so the
                            # first matmuls only queue behind ~2.5 MB of DMA
                            nc.sync.dma_start(
                                out=wq_cs[g], in_=wq_r[:, g * 4:(g + 1) * 4, :]
                            )
                            nc.sync.dma_start(
                                out=wk_cs[g], in_=wk_r[:, g * 4:(g + 1) * 4, :]
                            )
                            nc.sync.dma_start(
                                out=wv_cs[g], in_=wv_r[:, g * 4:(g + 1) * 4, :]
                            )
                        xt = xpool.tile([128, 4, 512], F32R, tag="xt")
                        nc.sync.dma_start(
                            out=xt, in_=xT_r[:, g * 4:(g + 1) * 4, ss]
                        )
                        for i in range(4):
                            kt = g * 4 + i
                            st = (kt == 0)
                            sp = (kt == NKT - 1)
                            for h in range(QH):
                                nc.tensor.matmul(
                                    q_ps[h],
                                    wq_cs[g][:, i, h * 128:(h + 1) * 128],
                                    xt[:, i, :],
                                    start=st, stop=sp,
                                )
                            nc.tensor.matmul(
                                k_ps, wk_cs[g][:, i, :], xt[:, i, :],
                                start=st, stop=sp,
                            )
                            nc.tensor.matmul(
                                v_ps, wv_cs[g][:, i, :], xt[:, i, :],
                                start=st, stop=sp,
                            )

                    # RoPE tables for this block
                    cos_t = cspool.tile([128, 512], F32, tag="cos")
                    nc.sync.dma_start(out=cos_t, in_=cosT[:, ss])
                    sin_t = cspool.tile([128, 512], F32, tag="sin")
                    nc.sync.dma_start(out=sin_t, in_=sinTs[:, ss])

                    def rope(dst, src_ps):
                        # ACT copies release the PSUM bank fast (the next seq
                        # block's matmuls wait on it) and produce both the
                        # straight and half-rotated views, so every DVE op is
                        # partition-aligned (cross-partition SBUF operands are
                        # rejected by the BIR verifier; PSUM->SBUF offset
                        # copies are fine)
                        v = ropetmp.tile([128, 512], F32, tag="v", name="v")
                        vr = ropetmp.tile([128, 512], F32, tag="vr", name="vr")
                        nc.scalar.copy(v, src_ps)
                        nc.scalar.copy(vr[0:64, :], src_ps[64:128, :])
                        nc.scalar.copy(vr[64:128, :], src_ps[0:64, :])
                        t = ropetmp.tile([128, 512], F32, tag="t", name="t")
                        u = ropetmp.tile([128, 512], F32, tag="u", name="u")
                        nc.vector.tensor_mul(t, v, cos_t)
                        nc.vector.tensor_mul(u, vr, sin_t)
                        nc.vector.tensor_add(dst, t, u)

                    # V^T -> V via PE transposes (ACT evicts, keeping DVE free
                    # for RoPE; issued first so the PE transposes overlap ropes)
                    vt_sb = vtb.tile([128, 512], F32R, tag="vt")
                    nc.scalar.copy(vt_sb, v_ps)
                    for j in range(4):
                        vt_ps = psVT.tile([128, 128], F32R, tag="vtp", name="vtp")
                        nc.tensor.transpose(
                            vt_ps, vt_sb[:, j * 128:(j + 1) * 128], ident_sb
                        )
                        nc.scalar.copy(vN[:, sb * 4 + j, :], vt_ps)

                    for h in range(QH):
                        rope(qT[:, h, ss], q_ps[h])
                    rope(kT[:, ss], k_ps)

            # ---------------- Phase B/C: attention + out projection ----------------
            with (
                tc.tile_pool(name="wopool", bufs=1) as wopool,
                tc.tile_pool(name="expp", bufs=8) as expp,
                tc.tile_pool(name="otp", bufs=2) as otp,
                tc.tile_pool(name="dsmall", bufs=2) as dsmall,
                tc.tile_pool(name="bcp", bufs=2) as bcp,
                tc.tile_pool(name="outev", bufs=3) as outev,
                tc.tile_pool(name="psS", bufs=2, space="PSUM") as psS,
                tc.tile_pool(name="psD", bufs=2, space="PSUM") as psD,
                tc.tile_pool(name="psOT", bufs=2, space="PSUM") as psOT,
                tc.tile_pool(name="psC", bufs=2, space="PSUM") as psC,
            ):
                wo_sb = wopool.tile([128, QH, DIM], F32R)
                wo_r = wo.rearrange("(h p) n -> p h n", p=128)
                for h in range(QH):
                    for c in range(2):
                        nc.sync.dma_start(
                            out=wo_sb[:, h, c * 2048:(c + 1) * 2048],
                            in_=wo_r[:, h, c * 2048:(c + 1) * 2048],
                        )

                LAG = 4  # D/AV matmuls trail the score stream by LAG blocks
                for qb in range(NSB):
                    qs = slice(qb * 512, (qb + 1) * 512)
                    n_kb = 4 * qb + 4
                    ot_sb = [None] * QH
                    for h in range(QH):
                        d_ps = psD.tile([1, 512], F32, tag="dps", name="dps")
                        ot_ps = psOT.tile([128, 512], F32, tag="otps", name="otps")
                        ess = [None] * n_kb

                        def drain(kb):
                            nc.tensor.matmul(
                                d_ps, onesc_sb, ess[kb],
                                start=(kb == 0), stop=(kb == n_kb - 1),
                            )
                            nc.tensor.matmul(
                                ot_ps, vN[:, kb, :], ess[kb],
                                start=(kb == 0), stop=(kb == n_kb - 1),
                            )

                        for kb in range(n_kb):
                            s_ps = psS.tile([128, 512], F32, tag="sps", name="sps")
                            nc.tensor.matmul(
                                s_ps,
                                kT[:, kb * 128:(kb + 1) * 128],
                                qT[:, h, qs],
                                start=True, stop=True,
                            )
                            j = kb - 4 * qb
                            if j >= 0:  # diagonal block: causal mask
                                nc.vector.tensor_add(
                                    s_ps, s_ps,
                                    stair_sb[:, 384 - 128 * j:896 - 128 * j],
                                )
                            es = expp.tile([128, 512], F32R, tag="es", name="es")
                            nc.scalar.activation(
                                es, s_ps, mybir.ActivationFunctionType.Exp,
                                scale=SCALE,
                            )
                            ess[kb] = es
                            if kb >= LAG:
                                drain(kb - LAG)
                        for kb in range(max(0, n_kb - LAG), n_kb):
                            drain(kb)
                        # normalize: O^T * (1/D) broadcast across partitions
                        rd = dsmall.tile([1, 512], F32R, tag="rd", name="rd")
                        with nc.allow_low_precision("f32r reciprocal for PE bcast"):
                            nc.vector.reciprocal(rd, d_ps)
                        bc_ps = psS.tile([128, 512], F32, tag="sps", name="bc")
                        nc.tensor.matmul(
                            bc_ps, onesr_sb, rd, start=True, stop=True
                        )
                        bc_sb = bcp.tile([128, 512], F32, tag="bcsb", name="bcsb")
                        nc.scalar.copy(bc_sb, bc_ps)
                        ot = otp.tile([128, 512], F32R, tag=f"ot{h}", name=f"ot{h}")
                        nc.vector.tensor_mul(ot, ot_ps, bc_sb)
                        ot_sb[h] = ot

                    # Phase C for this query block
                    for qc in range(4):
                        for nb in range(8):
                            o_ps = psC.tile([128, 512], F32, tag="ops", name="ops")
                            for h in range(QH):
                                nc.tensor.matmul(
                                    o_ps,
                                    ot_sb[h][:, qc * 128:(qc + 1) * 128],
                                    wo_sb[:, h, nb * 512:(nb + 1) * 512],
                                    start=(h == 0), stop=(h == QH - 1),
                                )
                            ob = outev.tile([128, 512], F32, tag="ob", name="ob")
                            nc.vector.tensor_copy(ob, o_ps)
                            nc.sync.dma_start(
                                out=out[qb * 512 + qc * 128:
                                        qb * 512 + (qc + 1) * 128,
                                        nb * 512:(nb + 1) * 512],
                                in_=ob,
                            )
    nc.finalize()
    return nc


_NC_CACHE = {}


def _get_nc():
    if "nc" not in _NC_CACHE:
        _NC_CACHE["nc"] = build_nc()
    return _NC_CACHE["nc"]


def _host_prep(x, cos, sin, mask, wq, wk, wv, wo):
    xT = np.ascontiguousarray(x[0].T.astype(np.float32))
    cosT = np.ascontiguousarray(cos[:, 0, :].T.astype(np.float32))
    sinT = sin[:, 0, :].T.astype(np.float32)
    sinTs = np.ascontiguousarray(
        np.concatenate([-sinT[:64], sinT[64:]], axis=0)
    )
    rr = np.arange(128, dtype=np.int64)[:, None]
    cc = np.arange(896, dtype=np.int64)[None, :]
    stair = np.where(rr <= cc - 384, 0.0, NEG).astype(np.float32)
    ident = np.eye(128, dtype=np.float32)
    ones_col = np.ones((128, 1), dtype=np.float32)
    ones_row = np.ones((1, 128), dtype=np.float32)

    in_maps = []
    for i in range(N_CORES):
        in_maps.append({
            "xT": xT,
            "wq": np.ascontiguousarray(wq[:, i * QS:(i + 1) * QS]),
            "wk": np.ascontiguousarray(wk[:, i * 128:(i + 1) * 128]),
            "wv": np.ascontiguousarray(wv[:, i * 128:(i + 1) * 128]),
            "wo": np.ascontiguousarray(wo[i * QS:(i + 1) * QS, :]),
            "cosT": cosT,
            "sinTs": sinTs,
            "stair": stair,
            "ident": ident,
            "ones_col": ones_col,
            "ones_row": ones_row,
        })
    return in_maps


def kernel(x, cos, sin, mask, wq, wk, wv, wo, _trace=False, _trace_kwargs=None):
    nc = _get_nc()
    in_maps = _host_prep(x, cos, sin, mask, wq, wk, wv, wo)
    res = run_bass_kernel_spmd(
        nc, in_maps, list(range(N_CORES)), trace=_trace,
        **(_trace_kwargs or {}),
    )
    partials = [res.results[i]["out"] for i in range(N_CORES)]
    full = np.sum(np.stack(partials, axis=0), axis=0, dtype=np.float64)
    out = full.astype(np.float32)[None, :, :]
    if _trace:
        return out, res
    return out



# revision 2
# speedup vs baseline: 1.3782x; 1.3782x over previous
"""GQA attention (SEQ=2048, DIM=4096, 32 Q heads / 8 KV heads, head_dim=128),
tensor-parallel over heads across 8 NeuronCores.

Each core owns 4 Q heads + 1 KV head: wq/wk/wv split column-wise, wo split
row-wise; each core produces a partial (2048, 4096) output that the host sums
(the all-reduce of row-parallel wo).

Per-core kernel, bf16 matmul operands (1 cyc/row at any free size, half the
DMA/SBUF/DVE traffic of f32; PSUM accumulation stays fp32):
  A) QKV projections as per-projection passes over a resident x super-tile
     (K pass, V pass, Q0..Q3 passes per 512-seq block) so each pass's RoPE /
     eviction runs on ACT+DVE underneath the next pass's PE stream.
  B) Flash attention per (head, 512-query block): S^T = K^T_blk.T @ Q^T,
     causal staircase on diagonal blocks, exp on ACT (scale folded),
     es accumulated into an f32 row-sum tile on DVE, AV drains lag the score
     stream; D broadcast across partitions via a ones-matrix matmul, 1/D via
     the fast custom-DVE reciprocal, O^T scaled on DVE.
  C) out = O^T.T @ wo accumulated over the 4 heads, evicted bf16 (split
     ACT/DVE) into a staging tile, one 1 MiB DMA per 128-row stripe.
"""

import numpy as np

import concourse.bacc as bacc
import concourse.tile as tile
from concourse import mybir
from concourse.bass_utils import run_bass_kernel_spmd

F32 = mybir.dt.float32
F32R = mybir.dt.float32r
BF16 = mybir.dt.bfloat16

DIM = 4096
SEQ = 2048
HEAD_DIM = 128
N_CORES = 8
QH = 4              # q heads per core
QS = QH * HEAD_DIM  # 512: wq column slice per core
NKT = DIM // 128    # 32 contraction tiles
NSB = SEQ // 512    # 4 sequence blocks
SCALE = 1.0 / float(np.sqrt(HEAD_DIM))
NEG = -1e9
LAG = 4             # AV matmuls trail the score stream by LAG blocks


def build_nc():
    nc = bacc.Bacc(trn_type="TRN2")

    xT = nc.declare_dram_parameter("xT", [DIM, SEQ], BF16, isOutput=False)
    wq = nc.declare_dram_parameter("wq", [DIM, QS], BF16, isOutput=False)
    wk = nc.declare_dram_parameter("wk", [DIM, HEAD_DIM], BF16, isOutput=False)
    wv = nc.declare_dram_parameter("wv", [DIM, HEAD_DIM], BF16, isOutput=False)
    wo = nc.declare_dram_parameter("wo", [QS, DIM], BF16, isOutput=False)
    cosT = nc.declare_dram_parameter("cosT", [HEAD_DIM, SEQ], BF16, isOutput=False)
    sinTs = nc.declare_dram_parameter("sinTs", [HEAD_DIM, SEQ], BF16, isOutput=False)
    stair = nc.declare_dram_parameter("stair", [128, 896], F32, isOutput=False)
    ident = nc.declare_dram_parameter("ident", [128, 128], BF16, isOutput=False)
    ones_mat = nc.declare_dram_parameter("ones_mat", [128, 128], F32R, isOutput=False)
    out = nc.declare_dram_parameter("out", [SEQ, DIM], BF16, isOutput=True)

    with tile.TileContext(nc) as tc:
        with (
            tc.tile_pool(name="persist", bufs=1) as persist,
            tc.tile_pool(name="resid", bufs=1) as resid,
        ):
            stair_sb = persist.tile([128, 896], F32)
            nc.sync.dma_start(out=stair_sb, in_=stair[:, :])
            ident_sb = persist.tile([128, 128], BF16)
            nc.sync.dma_start(out=ident_sb, in_=ident[:, :])
            ones_sb = persist.tile([128, 128], F32R)
            nc.sync.dma_start(out=ones_sb, in_=ones_mat[:, :])

            # resident activations
            qT = resid.tile([128, QH, SEQ], BF16)          # Q^T per head (d, s)
            kT = resid.tile([128, SEQ], BF16)              # K^T (d, s)
            vN = resid.tile([128, SEQ // 128, 128], BF16)  # V natural (k, d)

            # ---------------- Phase A: projections + RoPE ----------------
            with (
                tc.tile_pool(name="wpool", bufs=1) as wpool,
                tc.tile_pool(name="xpool", bufs=2) as xpool,
                tc.tile_pool(name="cspool", bufs=2) as cspool,
                tc.tile_pool(name="ropetmp", bufs=2) as ropetmp,
                tc.tile_pool(name="vtb", bufs=2) as vtb,
                tc.tile_pool(name="psA", bufs=1, space="PSUM") as psA,
                tc.tile_pool(name="psVT", bufs=2, space="PSUM") as psVT,
            ):
                wk_sb = wpool.tile([128, NKT, HEAD_DIM], BF16)
                wv_sb = wpool.tile([128, NKT, HEAD_DIM], BF16)
                wq_sb = wpool.tile([128, NKT, QS], BF16)
                wq_r = wq.rearrange("(t p) m -> p t m", p=128)
                wk_r = wk.rearrange("(t p) m -> p t m", p=128)
                wv_r = wv.rearrange("(t p) m -> p t m", p=128)
                xT_r = xT.rearrange("(t p) s -> p t s", p=128)

                nc.sync.dma_start(out=wk_sb, in_=wk_r[:, :, :])
                nc.sync.dma_start(out=wv_sb, in_=wv_r[:, :, :])
                for g in range(4):
                    nc.sync.dma_start(
                        out=wq_sb[:, g * 8:(g + 1) * 8, :],
                        in_=wq_r[:, g * 8:(g + 1) * 8, :],
                    )

                def rope(dst, src_ps, cos_t, sin_t):
                    # ACT copies evict PSUM fast and produce the straight and
                    # half-rotated views (partition-shifted reads are only
                    # legal on ACT); DVE runs the bf16 mul/mul/add at 2x rate.
                    v = ropetmp.tile([128, 512], BF16, tag="v", name="v")
                    vr = ropetmp.tile([128, 512], BF16, tag="vr", name="vr")
                    nc.scalar.copy(v, src_ps)
                    nc.scalar.copy(vr[0:64, :], src_ps[64:128, :])
                    nc.scalar.copy(vr[64:128, :], src_ps[0:64, :])
                    t = ropetmp.tile([128, 512], BF16, tag="t", name="t")
                    u = ropetmp.tile([128, 512], BF16, tag="u", name="u")
                    nc.vector.tensor_mul(t, v, cos_t)
                    nc.vector.tensor_mul(u, vr, sin_t)
                    nc.vector.tensor_add(dst, t, u)

                for sb in range(NSB):
                    ss = slice(sb * 512, (sb + 1) * 512)
                    xb = xpool.tile([128, NKT, 512], BF16, tag="xb", name="xb")
                    for g in range(8):
                        nc.sync.dma_start(
                            out=xb[:, g * 4:(g + 1) * 4, :],
                            in_=xT_r[:, g * 4:(g + 1) * 4, ss],
                        )
                    cos_t = cspool.tile([128, 512], BF16, tag="cos")
                    nc.sync.dma_start(out=cos_t, in_=cosT[:, ss])
                    sin_t = cspool.tile([128, 512], BF16, tag="sin")
                    nc.sync.dma_start(out=sin_t, in_=sinTs[:, ss])

                    # K pass, roped on ACT/DVE under the V pass
                    k_ps = psA.tile([128, 512], F32, tag="kps", name="kps")
                    for kt in range(NKT):
                        nc.tensor.matmul(
                            k_ps, wk_sb[:, kt, :], xb[:, kt, :],
                            start=(kt == 0), stop=(kt == NKT - 1),
                        )
                    rope(kT[:, ss], k_ps, cos_t, sin_t)

                    # V pass, evicted + transposed under the Q passes
                    v_ps = psA.tile([128, 512], F32, tag="vps", name="vps")
                    for kt in range(NKT):
                        nc.tensor.matmul(
                            v_ps, wv_sb[:, kt, :], xb[:, kt, :],
                            start=(kt == 0), stop=(kt == NKT - 1),
                        )
                    vt_sb = vtb.tile([128, 512], BF16, tag="vt", name="vt")
                    nc.scalar.copy(vt_sb, v_ps)

                    for h in range(QH):
                        q_ps = psA.tile([128, 512], F32, tag=f"qps{h}",
                                        name=f"qps{h}")
                        for kt in range(NKT):
                            nc.tensor.matmul(
                                q_ps, wq_sb[:, kt, h * 128:(h + 1) * 128],
                                xb[:, kt, :],
                                start=(kt == 0), stop=(kt == NKT - 1),
                            )
                        if h == 0:
                            # PE transposes of V^T -> V, slotted between Q
                            # passes so they never wait on the ACT evict
                            for j in range(4):
                                vt_ps = psVT.tile([128, 128], BF16, tag="vtp",
                                                  name="vtp")
                                nc.tensor.transpose(
                                    vt_ps, vt_sb[:, j * 128:(j + 1) * 128],
                                    ident_sb,
                                )
                                nc.scalar.copy(vN[:, sb * 4 + j, :], vt_ps)
                        rope(qT[:, h, ss], q_ps, cos_t, sin_t)

            # ---------------- Phase B/C: attention + out projection ----------------
            with (
                tc.tile_pool(name="wopool", bufs=1) as wopool,
                tc.tile_pool(name="expp", bufs=8) as expp,
                tc.tile_pool(name="esum", bufs=2) as esum,
                tc.tile_pool(name="rdp", bufs=2) as rdp,
                tc.tile_pool(name="otp", bufs=2) as otp,
                tc.tile_pool(name="stg", bufs=2) as stg,
                tc.tile_pool(name="psS", bufs=2, space="PSUM") as psS,
                tc.tile_pool(name="psD", bufs=2, space="PSUM") as psD,
                tc.tile_pool(name="psOT", bufs=2, space="PSUM") as psOT,
                tc.tile_pool(name="psC", bufs=2, space="PSUM") as psC,
            ):
                wo_sb = wopool.tile([128, QH, DIM], BF16)
                wo_r = wo.rearrange("(h p) n -> p h n", p=128)
                for h in range(QH):
                    nc.sync.dma_start(out=wo_sb[:, h, :], in_=wo_r[:, h, :])

                for qb in range(NSB):
                    qs = slice(qb * 512, (qb + 1) * 512)
                    n_kb = 4 * qb + 4
                    ot_sb = [None] * QH
                    for h in range(QH):
                        ot_ps = psOT.tile([128, 512], F32, tag="otps",
                                          name="otps")
                        es_sum = esum.tile([128, 512], F32R, tag="esum",
                                           name="esum")
                        ess = [None] * n_kb

                        def drain(kb):
                            nc.tensor.matmul(
                                ot_ps, vN[:, kb, :], ess[kb],
                                start=(kb == 0), stop=(kb == n_kb - 1),
                            )

                        for kb in range(n_kb):
                            s_ps = psS.tile([128, 512], F32, tag="sps",
                                            name="sps")
                            nc.tensor.matmul(
                                s_ps,
                                kT[:, kb * 128:(kb + 1) * 128],
                                qT[:, h, qs],
                                start=True, stop=True,
                            )
                            j = kb - 4 * qb
                            if j >= 0:  # diagonal block: causal staircase
                                nc.vector.tensor_add(
                                    s_ps, s_ps,
                                    stair_sb[:, 384 - 128 * j:896 - 128 * j],
                                )
                            es = expp.tile([128, 512], BF16, tag="es",
                                           name="es")
                            nc.scalar.activation(
                                es, s_ps, mybir.ActivationFunctionType.Exp,
                                scale=SCALE,
                            )
                            ess[kb] = es
                            if kb == 0:
                                nc.vector.tensor_copy(es_sum, es)
                            else:
                                nc.vector.tensor_add(es_sum, es_sum, es)
                            if kb >= LAG:
                                drain(kb - LAG)
                        for kb in range(max(0, n_kb - LAG), n_kb):
                            drain(kb)
                        # D broadcast across partitions (every output row of
                        # ones^T @ es_sum is the key-dim column sum), then the
                        # fast reciprocal and the O^T scale, all off the PE
                        # critical path
                        d_ps = psD.tile([128, 512], F32, tag="dps", name="dps")
                        nc.tensor.matmul(
                            d_ps, ones_sb, es_sum, start=True, stop=True,
                        )
                        rd = rdp.tile([128, 512], F32, tag="rd", name="rd")
                        nc.vector.reciprocal_approx_fast(out=rd, in_=d_ps)
                        ot = otp.tile([128, 512], BF16, tag=f"ot{h}",
                                      name=f"ot{h}")
                        nc.vector.tensor_mul(ot, ot_ps, rd)
                        ot_sb[h] = ot

                    # Phase C for this query block
                    for qc in range(4):
                        stg_t = stg.tile([128, DIM], BF16, tag="stg",
                                         name="stg")
                        for nb in range(8):
                            o_ps = psC.tile([128, 512], F32, tag="ops",
                                            name="ops")
                            for h in range(QH):
                                nc.tensor.matmul(
                                    o_ps,
                                    ot_sb[h][:, qc * 128:(qc + 1) * 128],
                                    wo_sb[:, h, nb * 512:(nb + 1) * 512],
                                    start=(h == 0), stop=(h == QH - 1),
                                )
                            dst = stg_t[:, nb * 512:(nb + 1) * 512]
                            if nb % 2 == 0:
                                nc.vector.tensor_copy(dst, o_ps)
                            else:
                                nc.scalar.copy(dst, o_ps)
                        nc.sync.dma_start(
                            out=out[qb * 512 + qc * 128:
                                    qb * 512 + (qc + 1) * 128, :],
                            in_=stg_t,
                        )
    nc.finalize()
    return nc


_NC_CACHE = {}


def _get_nc():
    if "nc" not in _NC_CACHE:
        _NC_CACHE["nc"] = build_nc()
    return _NC_CACHE["nc"]


def _host_prep(x, cos, sin, mask, wq, wk, wv, wo):
    import ml_dtypes

    bf16 = ml_dtypes.bfloat16
    xT = np.ascontiguousarray(x[0].T).astype(bf16)
    cosT = np.ascontiguousarray(cos[:, 0, :].T).astype(bf16)
    sinT = sin[:, 0, :].T.astype(np.float32)
    sinTs = np.ascontiguousarray(
        np.concatenate([-sinT[:64], sinT[64:]], axis=0)
    ).astype(bf16)
    rr = np.arange(128, dtype=np.int64)[:, None]
    cc = np.arange(896, dtype=np.int64)[None, :]
    stair = np.where(rr <= cc - 384, 0.0, NEG).astype(np.float32)
    ident = np.eye(128).astype(bf16)
    ones_mat = np.ones((128, 128), dtype=np.float32)

    in_maps = []
    for i in range(N_CORES):
        in_maps.append({
            "xT": xT,
            "wq": np.ascontiguousarray(wq[:, i * QS:(i + 1) * QS]).astype(bf16),
            "wk": np.ascontiguousarray(wk[:, i * 128:(i + 1) * 128]).astype(bf16),
            "wv": np.ascontiguousarray(wv[:, i * 128:(i + 1) * 128]).astype(bf16),
            "wo": np.ascontiguousarray(wo[i * QS:(i + 1) * QS, :]).astype(bf16),
            "cosT": cosT,
            "sinTs": sinTs,
            "stair": stair,
            "ident": ident,
            "ones_mat": ones_mat,
        })
    return in_maps


def kernel(x, cos, sin, mask, wq, wk, wv, wo, _trace=False, _trace_kwargs=None):
    nc = _get_nc()
    in_maps = _host_prep(x, cos, sin, mask, wq, wk, wv, wo)
    res = run_bass_kernel_spmd(
        nc, in_maps, list(range(N_CORES)), trace=_trace,
        **(_trace_kwargs or {}),
    )
    partials = [res.results[i]["out"] for i in range(N_CORES)]
    full = np.sum(
        np.stack([p.astype(np.float32) for p in partials], axis=0),
        axis=0, dtype=np.float64,
    )
    out = full.astype(np.float32)[None, :, :]
    if _trace:
        return out, res
    return out


# revision 3
# speedup vs baseline: 1.4650x; 1.0629x over previous
"""GQA attention (SEQ=2048, DIM=4096, 32 Q heads / 8 KV heads, head_dim=128),
tensor-parallel over heads across 8 NeuronCores.

Each core owns 4 Q heads + 1 KV head: wq/wk/wv split column-wise, wo split
row-wise; each core produces a partial (2048, 4096) output that the host sums
(the all-reduce of row-parallel wo).

Per-core kernel, bf16 matmul operands (1 cyc/row at any free size, half the
DMA/SBUF/DVE traffic of f32; PSUM accumulation stays fp32):
  A) QKV projections as per-projection passes over a resident x super-tile
     (K pass, V pass, Q0..Q3 passes per 512-seq block) so each pass's RoPE /
     eviction runs on ACT+DVE underneath the next pass's PE stream. DMA order
     puts the first x chunks right after wk so the PE starts ~5us in.
  B) Flash attention per (head, 512-query block): S^T = K^T_blk.T @ Q^T,
     exp on ACT (scale folded) straight off PSUM, causal staircase applied
     as a 0/1 bf16 multiply AFTER exp (keeps the S->exp chain DVE-free),
     es accumulated into an f32 row-sum tile on DVE, AV drains lag the score
     stream; each head's D/reciprocal/scale chain is deferred into the next
     head's score stream so the PE never waits on the exp tail. D is
     broadcast across partitions via a ones-matrix matmul and inverted with
     the fast custom-DVE reciprocal.
  C) out = O^T.T @ wo accumulated over the 4 heads, evicted bf16 (split
     ACT/DVE) into a staging tile, DMA'd out in half-stripes for overlap.
"""

import numpy as np

import concourse.bacc as bacc
import concourse.tile as tile
from concourse import mybir
from concourse.bass_utils import run_bass_kernel_spmd

F32 = mybir.dt.float32
F32R = mybir.dt.float32r
BF16 = mybir.dt.bfloat16

DIM = 4096
SEQ = 2048
HEAD_DIM = 128
N_CORES = 8
QH = 4              # q heads per core
QS = QH * HEAD_DIM  # 512: wq column slice per core
NKT = DIM // 128    # 32 contraction tiles
NSB = SEQ // 512    # 4 sequence blocks
SCALE = 1.0 / float(np.sqrt(HEAD_DIM))
LAG = 4             # AV matmuls trail the score stream by LAG blocks


def build_nc():
    nc = bacc.Bacc(trn_type="TRN2")

    xT = nc.declare_dram_parameter("xT", [DIM, SEQ], BF16, isOutput=False)
    wq = nc.declare_dram_parameter("wq", [DIM, QS], BF16, isOutput=False)
    wk = nc.declare_dram_parameter("wk", [DIM, HEAD_DIM], BF16, isOutput=False)
    wv = nc.declare_dram_parameter("wv", [DIM, HEAD_DIM], BF16, isOutput=False)
    wo = nc.declare_dram_parameter("wo", [QS, DIM], BF16, isOutput=False)
    cosT = nc.declare_dram_parameter("cosT", [HEAD_DIM, SEQ], BF16, isOutput=False)
    sinTs = nc.declare_dram_parameter("sinTs", [HEAD_DIM, SEQ], BF16, isOutput=False)
    stair = nc.declare_dram_parameter("stair", [128, 896], BF16, isOutput=False)
    ident = nc.declare_dram_parameter("ident", [128, 128], BF16, isOutput=False)
    ones_mat = nc.declare_dram_parameter("ones_mat", [128, 128], F32R, isOutput=False)
    out = nc.declare_dram_parameter("out", [SEQ, DIM], BF16, isOutput=True)

    with tile.TileContext(nc) as tc:
        with (
            tc.tile_pool(name="persist", bufs=1) as persist,
            tc.tile_pool(name="resid", bufs=1) as resid,
        ):
            stair_sb = persist.tile([128, 896], BF16)
            nc.sync.dma_start(out=stair_sb, in_=stair[:, :])
            ident_sb = persist.tile([128, 128], BF16)
            nc.sync.dma_start(out=ident_sb, in_=ident[:, :])
            ones_sb = persist.tile([128, 128], F32R)
            nc.sync.dma_start(out=ones_sb, in_=ones_mat[:, :])

            # resident activations
            qT = resid.tile([128, QH, SEQ], BF16)          # Q^T per head (d, s)
            kT = resid.tile([128, SEQ], BF16)              # K^T (d, s)
            vN = resid.tile([128, SEQ // 128, 128], BF16)  # V natural (k, d)

            # ---------------- Phase A: projections + RoPE ----------------
            with (
                tc.tile_pool(name="wpool", bufs=1) as wpool,
                tc.tile_pool(name="xpool", bufs=2) as xpool,
                tc.tile_pool(name="cspool", bufs=2) as cspool,
                tc.tile_pool(name="ropetmp", bufs=2) as ropetmp,
                tc.tile_pool(name="vtb", bufs=2) as vtb,
                tc.tile_pool(name="psA", bufs=1, space="PSUM") as psA,
                tc.tile_pool(name="psVT", bufs=2, space="PSUM") as psVT,
            ):
                wk_sb = wpool.tile([128, NKT, HEAD_DIM], BF16)
                wv_sb = wpool.tile([128, NKT, HEAD_DIM], BF16)
                wq_sb = wpool.tile([128, NKT, QS], BF16)
                wq_r = wq.rearrange("(t p) m -> p t m", p=128)
                wk_r = wk.rearrange("(t p) m -> p t m", p=128)
                wv_r = wv.rearrange("(t p) m -> p t m", p=128)
                xT_r = xT.rearrange("(t p) s -> p t s", p=128)

                # DMA issue order feeds the PE critical path: wk, then the
                # first seq block of x, then wv/cos/sin/wq (needed ~15+us in)
                nc.sync.dma_start(out=wk_sb, in_=wk_r[:, :, :])
                xb0 = xpool.tile([128, NKT, 512], BF16, tag="xb", name="xb")
                for g in range(8):
                    nc.sync.dma_start(
                        out=xb0[:, g * 4:(g + 1) * 4, :],
                        in_=xT_r[:, g * 4:(g + 1) * 4, 0:512],
                    )
                nc.sync.dma_start(out=wv_sb, in_=wv_r[:, :, :])
                cos0 = cspool.tile([128, 512], BF16, tag="cos")
                nc.sync.dma_start(out=cos0, in_=cosT[:, 0:512])
                sin0 = cspool.tile([128, 512], BF16, tag="sin")
                nc.sync.dma_start(out=sin0, in_=sinTs[:, 0:512])
                for g in range(4):
                    nc.sync.dma_start(
                        out=wq_sb[:, g * 8:(g + 1) * 8, :],
                        in_=wq_r[:, g * 8:(g + 1) * 8, :],
                    )

                def rope(dst, src_ps, cos_t, sin_t):
                    # ACT copies evict PSUM fast and produce the straight and
                    # half-rotated views (partition-shifted reads are only
                    # legal on ACT); DVE runs the bf16 mul/mul/add at 2x rate.
                    v = ropetmp.tile([128, 512], BF16, tag="v", name="v")
                    vr = ropetmp.tile([128, 512], BF16, tag="vr", name="vr")
                    nc.scalar.copy(v, src_ps)
                    nc.scalar.copy(vr[0:64, :], src_ps[64:128, :])
                    nc.scalar.copy(vr[64:128, :], src_ps[0:64, :])
                    t = ropetmp.tile([128, 512], BF16, tag="t", name="t")
                    u = ropetmp.tile([128, 512], BF16, tag="u", name="u")
                    nc.vector.tensor_mul(t, v, cos_t)
                    nc.vector.tensor_mul(u, vr, sin_t)
                    nc.vector.tensor_add(dst, t, u)

                for sb in range(NSB):
                    ss = slice(sb * 512, (sb + 1) * 512)
                    if sb == 0:
                        xb, cos_t, sin_t = xb0, cos0, sin0
                    else:
                        xb = xpool.tile([128, NKT, 512], BF16, tag="xb",
                                        name="xb")
                        for g in range(8):
                            nc.sync.dma_start(
                                out=xb[:, g * 4:(g + 1) * 4, :],
                                in_=xT_r[:, g * 4:(g + 1) * 4, ss],
                            )
                        cos_t = cspool.tile([128, 512], BF16, tag="cos")
                        nc.sync.dma_start(out=cos_t, in_=cosT[:, ss])
                        sin_t = cspool.tile([128, 512], BF16, tag="sin")
                        nc.sync.dma_start(out=sin_t, in_=sinTs[:, ss])

                    # K pass, roped on ACT/DVE under the V pass
                    k_ps = psA.tile([128, 512], F32, tag="kps", name="kps")
                    for kt in range(NKT):
                        nc.tensor.matmul(
                            k_ps, wk_sb[:, kt, :], xb[:, kt, :],
                            start=(kt == 0), stop=(kt == NKT - 1),
                        )
                    rope(kT[:, ss], k_ps, cos_t, sin_t)

                    # V pass, evicted + transposed under the Q passes
                    v_ps = psA.tile([128, 512], F32, tag="vps", name="vps")
                    for kt in range(NKT):
                        nc.tensor.matmul(
                            v_ps, wv_sb[:, kt, :], xb[:, kt, :],
                            start=(kt == 0), stop=(kt == NKT - 1),
                        )
                    vt_sb = vtb.tile([128, 512], BF16, tag="vt", name="vt")
                    nc.scalar.copy(vt_sb, v_ps)

                    for h in range(QH):
                        q_ps = psA.tile([128, 512], F32, tag=f"qps{h}",
                                        name=f"qps{h}")
                        for kt in range(NKT):
                            nc.tensor.matmul(
                                q_ps, wq_sb[:, kt, h * 128:(h + 1) * 128],
                                xb[:, kt, :],
                                start=(kt == 0), stop=(kt == NKT - 1),
                            )
                        if h == 0:
                            # PE transposes of V^T -> V, slotted between Q
                            # passes so they never wait on the ACT evict
                            for j in range(4):
                                vt_ps = psVT.tile([128, 128], BF16, tag="vtp",
                                                  name="vtp")
                                nc.tensor.transpose(
                                    vt_ps, vt_sb[:, j * 128:(j + 1) * 128],
                                    ident_sb,
                                )
                                nc.scalar.copy(vN[:, sb * 4 + j, :], vt_ps)
                        rope(qT[:, h, ss], q_ps, cos_t, sin_t)

            # ---------------- Phase B/C: attention + out projection ----------------
            with (
                tc.tile_pool(name="wopool", bufs=1) as wopool,
                tc.tile_pool(name="expp", bufs=10) as expp,
                tc.tile_pool(name="esum", bufs=2) as esum,
                tc.tile_pool(name="rdp", bufs=2) as rdp,
                tc.tile_pool(name="otp", bufs=2) as otp,
                tc.tile_pool(name="stg", bufs=2) as stg,
                tc.tile_pool(name="psS", bufs=2, space="PSUM") as psS,
                tc.tile_pool(name="psD", bufs=2, space="PSUM") as psD,
                tc.tile_pool(name="psOT", bufs=2, space="PSUM") as psOT,
                tc.tile_pool(name="psC", bufs=2, space="PSUM") as psC,
            ):
                wo_sb = wopool.tile([128, QH, DIM], BF16)
                wo_r = wo.rearrange("(h p) n -> p h n", p=128)
                for h in range(QH):
                    nc.sync.dma_start(out=wo_sb[:, h, :], in_=wo_r[:, h, :])

                for qb in range(NSB):
                    qs = slice(qb * 512, (qb + 1) * 512)
                    n_kb = 4 * qb + 4
                    ot_sb = [None] * QH
                    # deferred D/reciprocal/scale chain of the previous head,
                    # issued under the current head's score stream
                    pending = [None]

                    def flush_pending():
                        if pending[0] is not None:
                            pending[0]()
                            pending[0] = None

                    for h in range(QH):
                        ot_ps = psOT.tile([128, 512], F32, tag="otps",
                                          name="otps")
                        es_sum = esum.tile([128, 512], F32R, tag="esum",
                                           name="esum")
                        ess = [None] * n_kb

                        def drain(kb, ot_ps=ot_ps, ess=ess):
                            nc.tensor.matmul(
                                ot_ps, vN[:, kb, :], ess[kb],
                                start=(kb == 0), stop=(kb == n_kb - 1),
                            )

                        for kb in range(n_kb):
                            s_ps = psS.tile([128, 512], F32, tag="sps",
                                            name="sps")
                            nc.tensor.matmul(
                                s_ps,
                                kT[:, kb * 128:(kb + 1) * 128],
                                qT[:, h, qs],
                                start=True, stop=True,
                            )
                            if kb == 2:
                                flush_pending()
                            es = expp.tile([128, 512], BF16, tag="es",
                                           name="es")
                            nc.scalar.activation(
                                es, s_ps, mybir.ActivationFunctionType.Exp,
                                scale=SCALE,
                            )
                            j = kb - 4 * qb
                            if j >= 0:  # diagonal block: 0/1 causal staircase
                                esm = expp.tile([128, 512], BF16, tag="es",
                                                name="esm")
                                nc.vector.tensor_mul(
                                    esm, es,
                                    stair_sb[:, 384 - 128 * j:896 - 128 * j],
                                )
                                es = esm
                            ess[kb] = es
                            if kb == 0:
                                nc.vector.tensor_copy(es_sum, es)
                            else:
                                nc.vector.tensor_add(es_sum, es_sum, es)
                            if kb >= LAG:
                                drain(kb - LAG)
                        for kb in range(max(0, n_kb - LAG), n_kb):
                            drain(kb)
                        flush_pending()

                        def dchain(h=h, ot_ps=ot_ps, es_sum=es_sum):
                            # D broadcast across partitions (each output row
                            # of ones^T @ es_sum is the key-dim column sum),
                            # fast reciprocal, O^T scale
                            d_ps = psD.tile([128, 512], F32, tag="dps",
                                            name="dps")
                            nc.tensor.matmul(
                                d_ps, ones_sb, es_sum, start=True, stop=True,
                            )
                            rd = rdp.tile([128, 512], F32, tag="rd", name="rd")
                            nc.vector.reciprocal_approx_fast(out=rd, in_=d_ps)
                            ot = otp.tile([128, 512], BF16, tag=f"ot{h}",
                                          name=f"ot{h}")
                            nc.vector.tensor_mul(ot, ot_ps, rd)
                            ot_sb[h] = ot

                        pending[0] = dchain
                    flush_pending()

                    # Phase C for this query block
                    for qc in range(4):
                        stg_t = stg.tile([128, DIM], BF16, tag="stg",
                                         name="stg")
                        for nb in range(8):
                            o_ps = psC.tile([128, 512], F32, tag="ops",
                                            name="ops")
                            for h in range(QH):
                                nc.tensor.matmul(
                                    o_ps,
                                    ot_sb[h][:, qc * 128:(qc + 1) * 128],
                                    wo_sb[:, h, nb * 512:(nb + 1) * 512],
                                    start=(h == 0), stop=(h == QH - 1),
                                )
                            dst = stg_t[:, nb * 512:(nb + 1) * 512]
                            if nb % 2 == 0:
                                nc.vector.tensor_copy(dst, o_ps)
                            else:
                                nc.scalar.copy(dst, o_ps)
                            if nb == 3 or nb == 7:
                                half = slice((nb - 3) * 512, (nb + 1) * 512)
                                nc.sync.dma_start(
                                    out=out[qb * 512 + qc * 128:
                                            qb * 512 + (qc + 1) * 128, half],
                                    in_=stg_t[:, half],
                                )
    nc.finalize()
    return nc


_NC_CACHE = {}


def _get_nc():
    if "nc" not in _NC_CACHE:
        _NC_CACHE["nc"] = build_nc()
    return _NC_CACHE["nc"]


def _host_prep(x, cos, sin, mask, wq, wk, wv, wo):
    import ml_dtypes

    bf16 = ml_dtypes.bfloat16
    xT = np.ascontiguousarray(x[0].T).astype(bf16)
    cosT = np.ascontiguousarray(cos[:, 0, :].T).astype(bf16)
    sinT = sin[:, 0, :].T.astype(np.float32)
    sinTs = np.ascontiguousarray(
        np.concatenate([-sinT[:64], sinT[64:]], axis=0)
    ).astype(bf16)
    rr = np.arange(128, dtype=np.int64)[:, None]
    cc = np.arange(896, dtype=np.int64)[None, :]
    stair = np.where(rr <= cc - 384, 1.0, 0.0).astype(bf16)
    ident = np.eye(128).astype(bf16)
    ones_mat = np.ones((128, 128), dtype=np.float32)

    in_maps = []
    for i in range(N_CORES):
        in_maps.append({
            "xT": xT,
            "wq": np.ascontiguousarray(wq[:, i * QS:(i + 1) * QS]).astype(bf16),
            "wk": np.ascontiguousarray(wk[:, i * 128:(i + 1) * 128]).astype(bf16),
            "wv": np.ascontiguousarray(wv[:, i * 128:(i + 1) * 128]).astype(bf16),
            "wo": np.ascontiguousarray(wo[i * QS:(i + 1) * QS, :]).astype(bf16),
            "cosT": cosT,
            "sinTs": sinTs,
            "stair": stair,
            "ident": ident,
            "ones_mat": ones_mat,
        })
    return in_maps


def kernel(x, cos, sin, mask, wq, wk, wv, wo, _trace=False, _trace_kwargs=None):
    nc = _get_nc()
    in_maps = _host_prep(x, cos, sin, mask, wq, wk, wv, wo)
    res = run_bass_kernel_spmd(
        nc, in_maps, list(range(N_CORES)), trace=_trace,
        **(_trace_kwargs or {}),
    )
    partials = [res.results[i]["out"] for i in range(N_CORES)]
    full = np.sum(
        np.stack([p.astype(np.float32) for p in partials], axis=0),
        axis=0, dtype=np.float64,
    )
    out = full.astype(np.float32)[None, :, :]
    if _trace:
        return out, res
    return out


# revision 4
# speedup vs baseline: 1.4716x; 1.0045x over previous
"""GQA attention (SEQ=2048, DIM=4096, 32 Q heads / 8 KV heads, head_dim=128),
tensor-parallel over heads across 8 NeuronCores.

Each core owns 4 Q heads + 1 KV head: wq/wk/wv split column-wise, wo split
row-wise; each core produces a partial (2048, 4096) output that the host sums
(the all-reduce of row-parallel wo).

Per-core kernel, bf16 matmul operands (1 cyc/row at any free size, half the
DMA/SBUF/DVE traffic of f32; PSUM accumulation stays fp32):
  A) QKV projections as per-projection passes over per-chunk x tiles
     (K pass, V pass, Q0..Q3 passes per 512-seq block) so each pass's RoPE /
     eviction runs on ACT+DVE underneath the next pass's PE stream. All DMAs
     land in chunk-sized tiles so the first matmul waits on ~1 MiB, not the
     whole load.
  B/C) software-pipelined: the attention streams for query block qb carry
     the output projection of block qb-1 inside them. Per (head h, qb):
     S^T matmuls + exp (ACT, scale folded) + 0/1 staircase mask after exp
     (DVE) + row-sum accumulation (split GPSIMD/DVE) + lagged AV drains,
     with the wo-projection pair groups of (qb-1, qc=h) and the previous
     head's deferred tail drains / D-reciprocal chain interleaved between
     score matmuls. This keeps the PE fed while ACT works through the exp
     stream (exp is slower per block than the S+AV pair it feeds), and the
     D chain (ones-matrix broadcast matmul -> custom-DVE fast reciprocal ->
     O^T scale) always hides under later PE work.
"""

import numpy as np

import concourse.bacc as bacc
import concourse.tile as tile
from concourse import mybir
from concourse.bass_utils import run_bass_kernel_spmd

F32 = mybir.dt.float32
F32R = mybir.dt.float32r
BF16 = mybir.dt.bfloat16

DIM = 4096
SEQ = 2048
HEAD_DIM = 128
N_CORES = 8
QH = 4              # q heads per core
QS = QH * HEAD_DIM  # 512: wq column slice per core
NKT = DIM // 128    # 32 contraction tiles
NSB = SEQ // 512    # 4 sequence blocks
SCALE = 1.0 / float(np.sqrt(HEAD_DIM))
LAG = 4             # AV matmuls trail the score stream by LAG blocks


def build_nc():
    nc = bacc.Bacc(trn_type="TRN2")

    xT = nc.declare_dram_parameter("xT", [DIM, SEQ], BF16, isOutput=False)
    wq = nc.declare_dram_parameter("wq", [DIM, QS], BF16, isOutput=False)
    wk = nc.declare_dram_parameter("wk", [DIM, HEAD_DIM], BF16, isOutput=False)
    wv = nc.declare_dram_parameter("wv", [DIM, HEAD_DIM], BF16, isOutput=False)
    wo = nc.declare_dram_parameter("wo", [QS, DIM], BF16, isOutput=False)
    cosT = nc.declare_dram_parameter("cosT", [HEAD_DIM, SEQ], BF16, isOutput=False)
    sinTs = nc.declare_dram_parameter("sinTs", [HEAD_DIM, SEQ], BF16, isOutput=False)
    stair = nc.declare_dram_parameter("stair", [128, 896], BF16, isOutput=False)
    ident = nc.declare_dram_parameter("ident", [128, 128], BF16, isOutput=False)
    ones_mat = nc.declare_dram_parameter("ones_mat", [128, 128], F32R, isOutput=False)
    out = nc.declare_dram_parameter("out", [SEQ, DIM], BF16, isOutput=True)

    with tile.TileContext(nc) as tc:
        with (
            tc.tile_pool(name="persist", bufs=1) as persist,
            tc.tile_pool(name="resid", bufs=1) as resid,
        ):
            stair_sb = persist.tile([128, 896], BF16)
            nc.sync.dma_start(out=stair_sb, in_=stair[:, :])
            ident_sb = persist.tile([128, 128], BF16)
            nc.sync.dma_start(out=ident_sb, in_=ident[:, :])
            ones_sb = persist.tile([128, 128], F32R)
            nc.sync.dma_start(out=ones_sb, in_=ones_mat[:, :])

            # resident activations
            qT = resid.tile([128, QH, SEQ], BF16)          # Q^T per head (d, s)
            kT = resid.tile([128, SEQ], BF16)              # K^T (d, s)
            vN = resid.tile([128, SEQ // 128, 128], BF16)  # V natural (k, d)

            # ---------------- Phase A: projections + RoPE ----------------
            with (
                tc.tile_pool(name="wpool", bufs=1) as wpool,
                tc.tile_pool(name="xpool", bufs=2) as xpool,
                tc.tile_pool(name="cspool", bufs=2) as cspool,
                tc.tile_pool(name="ropetmp", bufs=2) as ropetmp,
                tc.tile_pool(name="vtb", bufs=2) as vtb,
                tc.tile_pool(name="psA", bufs=1, space="PSUM") as psA,
                tc.tile_pool(name="psVT", bufs=2, space="PSUM") as psVT,
            ):
                # per-chunk weight tiles: a matmul only waits on the one DMA
                # that feeds its chunk, not the whole weight load
                wk_cs = [wpool.tile([128, 16, HEAD_DIM], BF16, name=f"wk{c}")
                         for c in range(2)]
                wv_cs = [wpool.tile([128, 16, HEAD_DIM], BF16, name=f"wv{c}")
                         for c in range(2)]
                wq_cs = [wpool.tile([128, 8, QS], BF16, name=f"wq{c}")
                         for c in range(4)]
                wq_r = wq.rearrange("(t p) m -> p t m", p=128)
                wk_r = wk.rearrange("(t p) m -> p t m", p=128)
                wv_r = wv.rearrange("(t p) m -> p t m", p=128)
                xT_r = xT.rearrange("(t p) s -> p t s", p=128)

                def xchunks(sb):
                    xs = []
                    for g in range(8):
                        xg = xpool.tile([128, 4, 512], BF16, tag=f"xb{g}",
                                        name=f"xb{g}")
                        nc.sync.dma_start(
                            out=xg,
                            in_=xT_r[:, g * 4:(g + 1) * 4,
                                     sb * 512:(sb + 1) * 512],
                        )
                        xs.append(xg)
                    return xs

                # DMA issue order feeds the PE critical path
                nc.sync.dma_start(out=wk_cs[0], in_=wk_r[:, 0:16, :])
                xb = xchunks(0)
                nc.sync.dma_start(out=wk_cs[1], in_=wk_r[:, 16:32, :])
                nc.sync.dma_start(out=wv_cs[0], in_=wv_r[:, 0:16, :])
                nc.sync.dma_start(out=wv_cs[1], in_=wv_r[:, 16:32, :])
                cos_t = cspool.tile([128, 512], BF16, tag="cos")
                nc.sync.dma_start(out=cos_t, in_=cosT[:, 0:512])
                sin_t = cspool.tile([128, 512], BF16, tag="sin")
                nc.sync.dma_start(out=sin_t, in_=sinTs[:, 0:512])
                for c in range(4):
                    nc.sync.dma_start(
                        out=wq_cs[c], in_=wq_r[:, c * 8:(c + 1) * 8, :]
                    )

                def rope(dst, src_ps, cos_t, sin_t):
                    # ACT copies evict PSUM fast and produce the straight and
                    # half-rotated views (partition-shifted reads are only
                    # legal on ACT); DVE runs the bf16 mul/mul/add at 2x rate.
                    v = ropetmp.tile([128, 512], BF16, tag="v", name="v")
                    vr = ropetmp.tile([128, 512], BF16, tag="vr", name="vr")
                    nc.scalar.copy(v, src_ps)
                    nc.scalar.copy(vr[0:64, :], src_ps[64:128, :])
                    nc.scalar.copy(vr[64:128, :], src_ps[0:64, :])
                    t = ropetmp.tile([128, 512], BF16, tag="t", name="t")
                    u = ropetmp.tile([128, 512], BF16, tag="u", name="u")
                    nc.vector.tensor_mul(t, v, cos_t)
                    nc.vector.tensor_mul(u, vr, sin_t)
                    nc.vector.tensor_add(dst, t, u)

                for sb in range(NSB):
                    ss = slice(sb * 512, (sb + 1) * 512)
                    if sb > 0:
                        xb = xchunks(sb)
                        cos_t = cspool.tile([128, 512], BF16, tag="cos")
                        nc.sync.dma_start(out=cos_t, in_=cosT[:, ss])
                        sin_t = cspool.tile([128, 512], BF16, tag="sin")
                        nc.sync.dma_start(out=sin_t, in_=sinTs[:, ss])

                    # K pass, roped on ACT/DVE under the V pass
                    k_ps = psA.tile([128, 512], F32, tag="kps", name="kps")
                    for kt in range(NKT):
                        nc.tensor.matmul(
                            k_ps, wk_cs[kt // 16][:, kt % 16, :],
                            xb[kt // 4][:, kt % 4, :],
                            start=(kt == 0), stop=(kt == NKT - 1),
                        )
                    rope(kT[:, ss], k_ps, cos_t, sin_t)

                    # V pass, evicted + transposed under the Q passes
                    v_ps = psA.tile([128, 512], F32, tag="vps", name="vps")
                    for kt in range(NKT):
                        nc.tensor.matmul(
                            v_ps, wv_cs[kt // 16][:, kt % 16, :],
                            xb[kt // 4][:, kt % 4, :],
                            start=(kt == 0), stop=(kt == NKT - 1),
                        )
                    vt_sb = vtb.tile([128, 512], BF16, tag="vt", name="vt")
                    nc.scalar.copy(vt_sb, v_ps)

                    for h in range(QH):
                        q_ps = psA.tile([128, 512], F32, tag=f"qps{h}",
                                        name=f"qps{h}")
                        for kt in range(NKT):
                            nc.tensor.matmul(
                                q_ps,
                                wq_cs[kt // 8][:, kt % 8,
                                               h * 128:(h + 1) * 128],
                                xb[kt // 4][:, kt % 4, :],
                                start=(kt == 0), stop=(kt == NKT - 1),
                            )
                        if h == 0:
                            # PE transposes of V^T -> V, slotted between Q
                            # passes so they never wait on the ACT evict
                            for j in range(4):
                                vt_ps = psVT.tile([128, 128], BF16, tag="vtp",
                                                  name="vtp")
                                nc.tensor.transpose(
                                    vt_ps, vt_sb[:, j * 128:(j + 1) * 128],
                                    ident_sb,
                                )
                                nc.scalar.copy(vN[:, sb * 4 + j, :], vt_ps)
                        rope(qT[:, h, ss], q_ps, cos_t, sin_t)

            # ------- Phase B/C: attention with pipelined out projection -------
            with (
                tc.tile_pool(name="wopool", bufs=1) as wopool,
                tc.tile_pool(name="expp", bufs=24) as expp,
                tc.tile_pool(name="esum", bufs=2) as esum,
                tc.tile_pool(name="rdp", bufs=2) as rdp,
                tc.tile_pool(name="otp", bufs=2) as otp,
                tc.tile_pool(name="stg", bufs=2) as stg,
                tc.tile_pool(name="psS", bufs=2, space="PSUM") as psS,
                tc.tile_pool(name="psD", bufs=2, space="PSUM") as psD,
                tc.tile_pool(name="psOT", bufs=2, space="PSUM") as psOT,
                tc.tile_pool(name="psC", bufs=2, space="PSUM") as psC,
            ):
                wo_sbs = [wopool.tile([128, DIM], BF16, name=f"wo{h}")
                          for h in range(QH)]
                wo_r = wo.rearrange("(h p) n -> p h n", p=128)
                for h in range(QH):
                    nc.sync.dma_start(out=wo_sbs[h], in_=wo_r[:, h, :])

                ot_store = {}   # (qb, h) -> normalized O^T tile
                deferred = []   # closures: tail drains + D chains, issued
                                # under later PE work

                def flush(n=None):
                    k = len(deferred) if n is None else min(n, len(deferred))
                    for _ in range(k):
                        deferred.pop(0)()

                def c_items(qb, qc, last=False):
                    """Output-projection work of query block qb, row stripe
                    qc: 8 accumulation groups + evicts + 2 half-stripe DMAs,
                    as a list of closures."""
                    stg_t = stg.tile([128, DIM], BF16, tag="stg", name="stg")
                    items = []

                    def group(nb, evict_dve):
                        def go():
                            o_ps = psC.tile([128, 512], F32, tag="ops",
                                            name="ops")
                            for h in range(QH):
                                nc.tensor.matmul(
                                    o_ps,
                                    ot_store[(qb, h)][:, qc * 128:
                                                      (qc + 1) * 128],
                                    wo_sbs[h][:, nb * 512:(nb + 1) * 512],
                                    start=(h == 0), stop=(h == QH - 1),
                                )
                            dst = stg_t[:, nb * 512:(nb + 1) * 512]
                            if evict_dve:
                                nc.vector.tensor_copy(dst, o_ps)
                            else:
                                nc.scalar.copy(dst, o_ps)
                            if last:
                                nc.sync.dma_start(
                                    out=out[qb * 512 + qc * 128:
                                            qb * 512 + (qc + 1) * 128,
                                            nb * 512:(nb + 1) * 512],
                                    in_=dst,
                                )
                        return go

                    def dma(lo, hi):
                        def go():
                            nc.sync.dma_start(
                                out=out[qb * 512 + qc * 128:
                                        qb * 512 + (qc + 1) * 128,
                                        lo * 512:hi * 512],
                                in_=stg_t[:, lo * 512:hi * 512],
                            )
                        return go

                    for nb in range(8):
                        # during-stream evicts stay off ACT (it is running
                        # the exp stream); the trailing block alternates
                        items.append(group(nb, evict_dve=(not last) or
                                           nb % 2 == 0))
                        if not last:
                            if nb == 3:
                                items.append(dma(0, 4))
                            elif nb == 7:
                                items.append(dma(4, 8))
                    return items

                for qb in range(NSB):
                    qs = slice(qb * 512, (qb + 1) * 512)
                    n_kb = 4 * qb + 4
                    for h in range(QH):
                        citems = c_items(qb - 1, h) if qb > 0 else []
                        c_done = 0
                        ot_ps = psOT.tile([128, 512], F32, tag="otps",
                                          name="otps")
                        es_a = esum.tile([128, 512], F32R, tag="esa",
                                         name="esa")
                        es_b = esum.tile([128, 512], F32R, tag="esb",
                                         name="esb")
                        seen_a = [False]
                        seen_b = [False]
                        ess = [None] * n_kb

                        def drain(kb, ot_ps=ot_ps, ess=ess, n_kb=n_kb):
                            nc.tensor.matmul(
                                ot_ps, vN[:, kb, :], ess[kb],
                                start=(kb == 0), stop=(kb == n_kb - 1),
                            )

                        for kb in range(n_kb):
                            s_ps = psS.tile([128, 512], F32, tag="sps",
                                            name="sps")
                            nc.tensor.matmul(
                                s_ps,
                                kT[:, kb * 128:(kb + 1) * 128],
                                qT[:, h, qs],
                                start=True, stop=True,
                            )
                            es = expp.tile([128, 512], BF16, tag="es",
                                           name="es")
                            nc.scalar.activation(
                                es, s_ps, mybir.ActivationFunctionType.Exp,
                                scale=SCALE,
                            )
                            j = kb - 4 * qb
                            if j >= 0:  # diagonal: 0/1 causal staircase
                                esm = expp.tile([128, 512], BF16, tag="es",
                                                name="esm")
                                nc.vector.tensor_mul(
                                    esm, es,
                                    stair_sb[:, 384 - 128 * j:896 - 128 * j],
                                )
                                es = esm
                            ess[kb] = es
                            # row-sum accumulation split across the idle
                            # GPSIMD and the DVE
                            if kb % 3 < 2:
                                eng, acc, seen = nc.gpsimd, es_a, seen_a
                            else:
                                eng, acc, seen = nc.vector, es_b, seen_b
                            if not seen[0]:
                                eng.tensor_copy(acc, es)
                                seen[0] = True
                            else:
                                eng.tensor_add(acc, acc, es)
                            if kb >= LAG:
                                drain(kb - LAG)
                            # previous head's deferred chain, then this
                            # head's share of the qb-1 projection work
                            if kb == 0:
                                flush(2)
                            elif kb == 1:
                                flush(2)
                            elif kb == 2:
                                flush()
                            elif citems:
                                want = (len(citems) * (kb - 2)) // (n_kb - 3)
                                while c_done < want:
                                    citems[c_done]()
                                    c_done += 1
                        while c_done < len(citems):
                            citems[c_done]()
                            c_done += 1
                        for kb in range(max(0, n_kb - LAG), n_kb):
                            deferred.append(
                                lambda kb=kb, drain=drain: drain(kb)
                            )

                        def dchain(qb=qb, h=h, ot_ps=ot_ps, es_a=es_a,
                                   es_b=es_b):
                            # D broadcast across partitions (each output row
                            # of ones^T @ es_sum is the key-dim column sum),
                            # fast reciprocal, O^T scale
                            d_ps = psD.tile([128, 512], F32, tag="dps",
                                            name="dps")
                            nc.tensor.matmul(d_ps, ones_sb, es_a,
                                             start=True, stop=False)
                            nc.tensor.matmul(d_ps, ones_sb, es_b,
                                             start=False, stop=True)
                            rd = rdp.tile([128, 512], F32, tag="rd",
                                          name="rd")
                            nc.vector.reciprocal_approx_fast(out=rd, in_=d_ps)
                            ot = otp.tile([128, 512], BF16, tag=f"ot{h}",
                                          name=f"ot{h}")
                            nc.vector.tensor_mul(ot, ot_ps, rd)
                            ot_store[(qb, h)] = ot

                        deferred.append(dchain)

                # trailing projection of the last query block
                flush()
                for qc in range(QH):
                    for it in c_items(NSB - 1, qc, last=True):
                        it()
    nc.finalize()
    return nc


_NC_CACHE = {}


def _get_nc():
    if "nc" not in _NC_CACHE:
        _NC_CACHE["nc"] = build_nc()
    return _NC_CACHE["nc"]


def _host_prep(x, cos, sin, mask, wq, wk, wv, wo):
    import ml_dtypes

    bf16 = ml_dtypes.bfloat16
    xT = np.ascontiguousarray(x[0].T).astype(bf16)
    cosT = np.ascontiguousarray(cos[:, 0, :].T).astype(bf16)
    sinT = sin[:, 0, :].T.astype(np.float32)
    sinTs = np.ascontiguousarray(
        np.concatenate([-sinT[:64], sinT[64:]], axis=0)
    ).astype(bf16)
    rr = np.arange(128, dtype=np.int64)[:, None]
    cc = np.arange(896, dtype=np.int64)[None, :]
    stair = np.where(rr <= cc - 384, 1.0, 0.0).astype(bf16)
    ident = np.eye(128).astype(bf16)
    ones_mat = np.ones((128, 128), dtype=np.float32)

    in_maps = []
    for i in range(N_CORES):
        in_maps.append({
            "xT": xT,
            "wq": np.ascontiguousarray(wq[:, i * QS:(i + 1) * QS]).astype(bf16),
            "wk": np.ascontiguousarray(wk[:, i * 128:(i + 1) * 128]).astype(bf16),
            "wv": np.ascontiguousarray(wv[:, i * 128:(i + 1) * 128]).astype(bf16),
            "wo": np.ascontiguousarray(wo[i * QS:(i + 1) * QS, :]).astype(bf16),
            "cosT": cosT,
            "sinTs": sinTs,
            "stair": stair,
            "ident": ident,
            "ones_mat": ones_mat,
        })
    return in_maps


def kernel(x, cos, sin, mask, wq, wk, wv, wo, _trace=False, _trace_kwargs=None):
    nc = _get_nc()
    in_maps = _host_prep(x, cos, sin, mask, wq, wk, wv, wo)
    res = run_bass_kernel_spmd(
        nc, in_maps, list(range(N_CORES)), trace=_trace,
        **(_trace_kwargs or {}),
    )
    partials = [res.results[i]["out"] for i in range(N_CORES)]
    full = np.sum(
        np.stack([p.astype(np.float32) for p in partials], axis=0),
        axis=0, dtype=np.float64,
    )
    out = full.astype(np.float32)[None, :, :]
    if _trace:
        return out, res
    return out


# revision 11
# speedup vs baseline: 1.5091x; 1.0255x over previous
"""GQA attention (SEQ=2048, DIM=4096, 32 Q heads / 8 KV heads, head_dim=128),
tensor-parallel over heads across 8 NeuronCores.

Each core owns 4 Q heads + 1 KV head: wq/wk/wv split column-wise, wo split
row-wise; each core produces a partial (2048, 4096) output that the host sums
(the all-reduce of row-parallel wo).

Per-core kernel, bf16 matmul operands (1 cyc/row at any free size, half the
DMA/SBUF/DVE traffic of f32; PSUM accumulation stays fp32):
  A) QKV projections as per-projection passes over per-chunk x tiles
     (K pass, V pass, Q0..Q3 passes per 512-seq block) so each pass's RoPE /
     eviction runs on ACT+DVE underneath the next pass's PE stream. All DMAs
     land in chunk-sized tiles so the first matmul waits on ~1 MiB, not the
     whole load.
  B/C) software-pipelined: the attention streams for query block qb carry
     the output projection of block qb-1 inside them. Per (head h, qb):
     S^T matmuls + exp (ACT, scale folded) + 0/1 staircase mask after exp
     (DVE) + row-sum accumulation (split GPSIMD/DVE) + lagged AV drains,
     with the wo-projection pair groups of (qb-1, qc=h) and the previous
     head's deferred tail drains / D-reciprocal chain interleaved between
     score matmuls. This keeps the PE fed while ACT works through the exp
     stream (exp is slower per block than the S+AV pair it feeds), and the
     D chain (ones-matrix broadcast matmul -> custom-DVE fast reciprocal ->
     O^T scale) always hides under later PE work.
"""

import numpy as np

import concourse.bacc as bacc
import concourse.tile as tile
from concourse import mybir
from concourse.bass_utils import run_bass_kernel_spmd

F32 = mybir.dt.float32
F32R = mybir.dt.float32r
BF16 = mybir.dt.bfloat16

DIM = 4096
SEQ = 2048
HEAD_DIM = 128
N_CORES = 8
QH = 4              # q heads per core
QS = QH * HEAD_DIM  # 512: wq column slice per core
NKT = DIM // 128    # 32 contraction tiles
NSB = SEQ // 512    # 4 sequence blocks
SCALE = 1.0 / float(np.sqrt(HEAD_DIM))
LAG = 4             # AV matmuls trail the score stream by LAG blocks


def build_nc():
    nc = bacc.Bacc(trn_type="TRN2")

    xT = nc.declare_dram_parameter("xT", [DIM, SEQ], BF16, isOutput=False)
    wq = nc.declare_dram_parameter("wq", [DIM, QS], BF16, isOutput=False)
    wk = nc.declare_dram_parameter("wk", [DIM, HEAD_DIM], BF16, isOutput=False)
    wv = nc.declare_dram_parameter("wv", [DIM, HEAD_DIM], BF16, isOutput=False)
    wo = nc.declare_dram_parameter("wo", [QS, DIM], BF16, isOutput=False)
    cosT = nc.declare_dram_parameter("cosT", [HEAD_DIM, SEQ], BF16, isOutput=False)
    sinTs = nc.declare_dram_parameter("sinTs", [HEAD_DIM, SEQ], BF16, isOutput=False)
    stair = nc.declare_dram_parameter("stair", [128, 896], BF16, isOutput=False)
    ident = nc.declare_dram_parameter("ident", [128, 128], BF16, isOutput=False)
    ones_mat = nc.declare_dram_parameter("ones_mat", [128, 128], F32R, isOutput=False)
    out = nc.declare_dram_parameter("out", [SEQ, DIM], BF16, isOutput=True)

    with tile.TileContext(nc) as tc:
        with (
            tc.tile_pool(name="persist", bufs=1) as persist,
            tc.tile_pool(name="resid", bufs=1) as resid,
        ):
            stair_sb = persist.tile([128, 896], BF16)
            nc.sync.dma_start(out=stair_sb, in_=stair[:, :])
            ident_sb = persist.tile([128, 128], BF16)
            nc.sync.dma_start(out=ident_sb, in_=ident[:, :])
            ones_sb = persist.tile([128, 128], F32R)
            nc.sync.dma_start(out=ones_sb, in_=ones_mat[:, :])

            # resident activations; qT split per head so phase B's first
            # score matmuls only wait on their own head's RoPE
            qTs = [resid.tile([128, SEQ], BF16, name=f"qT{h}")
                   for h in range(QH)]
            kT = resid.tile([128, SEQ], BF16)              # K^T (d, s)
            vN = resid.tile([128, SEQ // 128, 128], BF16)  # V natural (k, d)

            # ---------------- Phase A: projections + RoPE ----------------
            with (
                tc.tile_pool(name="wpool", bufs=1) as wpool,
                tc.tile_pool(name="xpool", bufs=2) as xpool,
                tc.tile_pool(name="cspool", bufs=2) as cspool,
                tc.tile_pool(name="ropetmp", bufs=2) as ropetmp,
                tc.tile_pool(name="vtb", bufs=2) as vtb,
                tc.tile_pool(name="psA", bufs=1, space="PSUM") as psA,
                tc.tile_pool(name="psVT", bufs=2, space="PSUM") as psVT,
            ):
                # per-chunk weight tiles: a matmul only waits on the one DMA
                # that feeds its chunk, not the whole weight load
                wk_cs = [wpool.tile([128, 16, HEAD_DIM], BF16, name=f"wk{c}")
                         for c in range(2)]
                wv_cs = [wpool.tile([128, 16, HEAD_DIM], BF16, name=f"wv{c}")
                         for c in range(2)]
                wq_cs = [wpool.tile([128, 8, QS], BF16, name=f"wq{c}")
                         for c in range(4)]
                wq_r = wq.rearrange("(t p) m -> p t m", p=128)
                wk_r = wk.rearrange("(t p) m -> p t m", p=128)
                wv_r = wv.rearrange("(t p) m -> p t m", p=128)
                xT_r = xT.rearrange("(t p) s -> p t s", p=128)

                def xchunks(sb):
                    xs = []
                    for g in range(8):
                        xg = xpool.tile([128, 4, 512], BF16, tag=f"xb{g}",
                                        name=f"xb{g}")
                        nc.sync.dma_start(
                            out=xg,
                            in_=xT_r[:, g * 4:(g + 1) * 4,
                                     sb * 512:(sb + 1) * 512],
                        )
                        xs.append(xg)
                    return xs

                # DMA issue order feeds the PE critical path: the interleaved
                # K||V pass consumes x chunks at roughly DMA delivery rate
                nc.sync.dma_start(out=wk_cs[0], in_=wk_r[:, 0:16, :])
                nc.sync.dma_start(out=wv_cs[0], in_=wv_r[:, 0:16, :])
                xb = xchunks(0)
                nc.sync.dma_start(out=wk_cs[1], in_=wk_r[:, 16:32, :])
                nc.sync.dma_start(out=wv_cs[1], in_=wv_r[:, 16:32, :])
                cos_t = cspool.tile([128, 512], BF16, tag="cos")
                nc.sync.dma_start(out=cos_t, in_=cosT[:, 0:512])
                sin_t = cspool.tile([128, 512], BF16, tag="sin")
                nc.sync.dma_start(out=sin_t, in_=sinTs[:, 0:512])
                for c in range(4):
                    nc.sync.dma_start(
                        out=wq_cs[c], in_=wq_r[:, c * 8:(c + 1) * 8, :]
                    )

                def rope(dst, src_ps, cos_t, sin_t):
                    # ACT copies evict PSUM fast and produce the straight and
                    # half-rotated views (partition-shifted reads are only
                    # legal on ACT); DVE runs the bf16 mul/mul/add at 2x rate.
                    v = ropetmp.tile([128, 512], BF16, tag="v", name="v")
                    vr = ropetmp.tile([128, 512], BF16, tag="vr", name="vr")
                    nc.scalar.copy(v, src_ps)
                    nc.scalar.copy(vr[0:64, :], src_ps[64:128, :])
                    nc.scalar.copy(vr[64:128, :], src_ps[0:64, :])
                    t = ropetmp.tile([128, 512], BF16, tag="t", name="t")
                    u = ropetmp.tile([128, 512], BF16, tag="u", name="u")
                    nc.vector.tensor_mul(t, v, cos_t)
                    nc.vector.tensor_mul(u, vr, sin_t)
                    nc.vector.tensor_add(dst, t, u)

                for sb in range(NSB):
                    ss = slice(sb * 512, (sb + 1) * 512)
                    if sb > 0:
                        xb = xchunks(sb)
                        cos_t = cspool.tile([128, 512], BF16, tag="cos")
                        nc.sync.dma_start(out=cos_t, in_=cosT[:, ss])
                        sin_t = cspool.tile([128, 512], BF16, tag="sin")
                        nc.sync.dma_start(out=sin_t, in_=sinTs[:, ss])

                    # interleaved K||V pass (each x chunk feeds both, so the
                    # PE tracks the x DMA stream); rope-K and the V evict
                    # then run on ACT/DVE under the Q passes
                    k_ps = psA.tile([128, 512], F32, tag="kps", name="kps")
                    v_ps = psA.tile([128, 512], F32, tag="vps", name="vps")
                    for kt in range(NKT):
                        nc.tensor.matmul(
                            k_ps, wk_cs[kt // 16][:, kt % 16, :],
                            xb[kt // 4][:, kt % 4, :],
                            start=(kt == 0), stop=(kt == NKT - 1),
                        )
                        nc.tensor.matmul(
                            v_ps, wv_cs[kt // 16][:, kt % 16, :],
                            xb[kt // 4][:, kt % 4, :],
                            start=(kt == 0), stop=(kt == NKT - 1),
                        )
                    rope(kT[:, ss], k_ps, cos_t, sin_t)
                    vt_sb = vtb.tile([128, 512], BF16, tag="vt", name="vt")
                    nc.scalar.copy(vt_sb, v_ps)

                    for h in range(QH):
                        q_ps = psA.tile([128, 512], F32, tag=f"qps{h}",
                                        name=f"qps{h}")
                        for kt in range(NKT):
                            nc.tensor.matmul(
                                q_ps,
                                wq_cs[kt // 8][:, kt % 8,
                                               h * 128:(h + 1) * 128],
                                xb[kt // 4][:, kt % 4, :],
                                start=(kt == 0), stop=(kt == NKT - 1),
                            )
                        if h == 0:
                            # PE transposes of V^T -> V, slotted between Q
                            # passes so they never wait on the ACT evict
                            for j in range(4):
                                vt_ps = psVT.tile([128, 128], BF16, tag="vtp",
                                                  name="vtp")
                                nc.tensor.transpose(
                                    vt_ps, vt_sb[:, j * 128:(j + 1) * 128],
                                    ident_sb,
                                )
                                nc.scalar.copy(vN[:, sb * 4 + j, :], vt_ps)
                        rope(qTs[h][:, ss], q_ps, cos_t, sin_t)

            # ------- Phase B/C: attention with pipelined out projection -------
            with (
                tc.tile_pool(name="wopool", bufs=1) as wopool,
                tc.tile_pool(name="expp", bufs=24) as expp,
                tc.tile_pool(name="esum", bufs=2) as esum,
                tc.tile_pool(name="rdp", bufs=2) as rdp,
                tc.tile_pool(name="otp", bufs=2) as otp,
                tc.tile_pool(name="stg", bufs=2) as stg,
                tc.tile_pool(name="psS", bufs=2, space="PSUM") as psS,
                tc.tile_pool(name="psD", bufs=2, space="PSUM") as psD,
                tc.tile_pool(name="psOT", bufs=2, space="PSUM") as psOT,
                tc.tile_pool(name="psC", bufs=2, space="PSUM") as psC,
            ):
                wo_sbs = [wopool.tile([128, DIM], BF16, name=f"wo{h}")
                          for h in range(QH)]
                wo_r = wo.rearrange("(h p) n -> p h n", p=128)
                for h in range(QH):
                    nc.sync.dma_start(out=wo_sbs[h], in_=wo_r[:, h, :])

                ot_store = {}   # (qb, h) -> normalized O^T tile
                deferred = []   # closures: tail drains + D chains, issued
                                # under later PE work

                def flush(n=None):
                    k = len(deferred) if n is None else min(n, len(deferred))
                    for _ in range(k):
                        deferred.pop(0)()

                def c_items(qb, qc, last=False):
                    """Output-projection work of query block qb, row stripe
                    qc: 8 accumulation groups + evicts + 2 half-stripe DMAs,
                    as a list of closures."""
                    stg_t = stg.tile([128, DIM], BF16, tag="stg", name="stg")
                    items = []

                    def group(nb, evict_dve):
                        def go():
                            o_ps = psC.tile([128, 512], F32, tag="ops",
                                            name="ops")
                            for h in range(QH):
                                nc.tensor.matmul(
                                    o_ps,
                                    ot_store[(qb, h)][:, qc * 128:
                                                      (qc + 1) * 128],
                                    wo_sbs[h][:, nb * 512:(nb + 1) * 512],
                                    start=(h == 0), stop=(h == QH - 1),
                                )
                            dst = stg_t[:, nb * 512:(nb + 1) * 512]
                            if evict_dve:
                                nc.vector.tensor_copy(dst, o_ps)
                            else:
                                nc.scalar.copy(dst, o_ps)
                            if last:
                                nc.sync.dma_start(
                                    out=out[qb * 512 + qc * 128:
                                            qb * 512 + (qc + 1) * 128,
                                            nb * 512:(nb + 1) * 512],
                                    in_=dst,
                                )
                        return go

                    def dma(lo, hi):
                        def go():
                            nc.sync.dma_start(
                                out=out[qb * 512 + qc * 128:
                                        qb * 512 + (qc + 1) * 128,
                                        lo * 512:hi * 512],
                                in_=stg_t[:, lo * 512:hi * 512],
                            )
                        return go

                    for nb in range(8):
                        # during-stream evicts lean on DVE (ACT is running
                        # the exp stream); the trailing block alternates
                        items.append(group(nb, evict_dve=(nb < 5) if not last
                                           else nb % 2 == 0))
                        if not last:
                            if nb == 3:
                                items.append(dma(0, 4))
                            elif nb == 7:
                                items.append(dma(4, 8))
                    return items

                for qb in range(NSB):
                    qs = slice(qb * 512, (qb + 1) * 512)
                    n_kb = 4 * qb + 4
                    for h in range(QH):
                        citems = c_items(qb - 1, h) if qb > 0 else []
                        c_done = 0
                        ot_ps = psOT.tile([128, 512], F32, tag="otps",
                                          name="otps")
                        es_a = esum.tile([128, 512], F32R, tag="esa",
                                         name="esa")
                        es_b = esum.tile([128, 512], F32R, tag="esb",
                                         name="esb")
                        seen_a = [False]
                        seen_b = [False]
                        ess = [None] * n_kb

                        def drain(kb, ot_ps=ot_ps, ess=ess, n_kb=n_kb):
                            nc.tensor.matmul(
                                ot_ps, vN[:, kb, :], ess[kb],
                                start=(kb == 0), stop=(kb == n_kb - 1),
                            )

                        for kb in range(n_kb):
                            s_ps = psS.tile([128, 512], F32, tag="sps",
                                            name="sps")
                            nc.tensor.matmul(
                                s_ps,
                                kT[:, kb * 128:(kb + 1) * 128],
                                qTs[h][:, qs],
                                start=True, stop=True,
                            )
                            es = expp.tile([128, 512], BF16, tag="es",
                                           name="es")
                            nc.scalar.activation(
                                es, s_ps, mybir.ActivationFunctionType.Exp,
                                scale=SCALE,
                            )
                            j = kb - 4 * qb
                            if j >= 0:  # diagonal: 0/1 causal staircase
                                esm = expp.tile([128, 512], BF16, tag="es",
                                                name="esm")
                                nc.vector.tensor_mul(
                                    esm, es,
                                    stair_sb[:, 384 - 128 * j:896 - 128 * j],
                                )
                                es = esm
                            ess[kb] = es
                            # row-sum accumulation split GPSIMD/DVE (GPSIMD
                            # tensor ops run ~1.2us each, so it only gets a
                            # third of the stream)
                            if kb % 3 == 0:
                                eng, acc, seen = nc.gpsimd, es_a, seen_a
                            else:
                                eng, acc, seen = nc.vector, es_b, seen_b
                            if not seen[0]:
                                eng.tensor_copy(acc, es)
                                seen[0] = True
                            else:
                                eng.tensor_add(acc, acc, es)
                            if kb >= LAG:
                                drain(kb - LAG)
                            # previous head's deferred chain, then this
                            # head's share of the qb-1 projection work
                            if kb == 0:
                                flush(2)
                            elif kb == 1:
                                flush(2)
                            elif kb == 2:
                                flush()
                            elif citems:
                                want = (len(citems) * (kb - 2)) // (n_kb - 3)
                                while c_done < want:
                                    citems[c_done]()
                                    c_done += 1
                        while c_done < len(citems):
                            citems[c_done]()
                            c_done += 1
                        for kb in range(max(0, n_kb - LAG), n_kb):
                            deferred.append(
                                lambda kb=kb, drain=drain: drain(kb)
                            )

                        def dchain(qb=qb, h=h, ot_ps=ot_ps, es_a=es_a,
                                   es_b=es_b):
                            # D broadcast across partitions (each output row
                            # of ones^T @ es_sum is the key-dim column sum),
                            # fast reciprocal, O^T scale
                            d_ps = psD.tile([128, 512], F32, tag="dps",
                                            name="dps")
                            nc.tensor.matmul(d_ps, ones_sb, es_a,
                                             start=True, stop=False)
                            nc.tensor.matmul(d_ps, ones_sb, es_b,
                                             start=False, stop=True)
                            rd = rdp.tile([128, 512], F32, tag="rd",
                                          name="rd")
                            nc.vector.reciprocal_approx_fast(out=rd, in_=d_ps)
                            ot = otp.tile([128, 512], BF16, tag=f"ot{h}",
                                          name=f"ot{h}")
                            nc.vector.tensor_mul(ot, ot_ps, rd)
                            ot_store[(qb, h)] = ot

                        deferred.append(dchain)

                # trailing projection of the last query block
                flush()
                for qc in range(QH):
                    for it in c_items(NSB - 1, qc, last=True):
                        it()
    nc.finalize()
    return nc


_NC_CACHE = {}


def _get_nc():
    if "nc" not in _NC_CACHE:
        _NC_CACHE["nc"] = build_nc()
    return _NC_CACHE["nc"]


def _host_prep(x, cos, sin, mask, wq, wk, wv, wo):
    import ml_dtypes

    bf16 = ml_dtypes.bfloat16
    xT = np.ascontiguousarray(x[0].T).astype(bf16)
    cosT = np.ascontiguousarray(cos[:, 0, :].T).astype(bf16)
    sinT = sin[:, 0, :].T.astype(np.float32)
    sinTs = np.ascontiguousarray(
        np.concatenate([-sinT[:64], sinT[64:]], axis=0)
    ).astype(bf16)
    rr = np.arange(128, dtype=np.int64)[:, None]
    cc = np.arange(896, dtype=np.int64)[None, :]
    stair = np.where(rr <= cc - 384, 1.0, 0.0).astype(bf16)
    ident = np.eye(128).astype(bf16)
    ones_mat = np.ones((128, 128), dtype=np.float32)

    in_maps = []
    for i in range(N_CORES):
        in_maps.append({
            "xT": xT,
            "wq": np.ascontiguousarray(wq[:, i * QS:(i + 1) * QS]).astype(bf16),
            "wk": np.ascontiguousarray(wk[:, i * 128:(i + 1) * 128]).astype(bf16),
            "wv": np.ascontiguousarray(wv[:, i * 128:(i + 1) * 128]).astype(bf16),
            "wo": np.ascontiguousarray(wo[i * QS:(i + 1) * QS, :]).astype(bf16),
            "cosT": cosT,
            "sinTs": sinTs,
            "stair": stair,
            "ident": ident,
            "ones_mat": ones_mat,
        })
    return in_maps


def kernel(x, cos, sin, mask, wq, wk, wv, wo, _trace=False, _trace_kwargs=None):
    nc = _get_nc()
    in_maps = _host_prep(x, cos, sin, mask, wq, wk, wv, wo)
    res = run_bass_kernel_spmd(
        nc, in_maps, list(range(N_CORES)), trace=_trace,
        **(_trace_kwargs or {}),
    )
    partials = [res.results[i]["out"] for i in range(N_CORES)]
    full = np.sum(
        np.stack([p.astype(np.float32) for p in partials], axis=0),
        axis=0, dtype=np.float64,
    )
    out = full.astype(np.float32)[None, :, :]
    if _trace:
        return out, res
    return out


# revision 12
# speedup vs baseline: 1.5221x; 1.0086x over previous
"""GQA attention (SEQ=2048, DIM=4096, 32 Q heads / 8 KV heads, head_dim=128),
tensor-parallel over heads across 8 NeuronCores.

Each core owns 4 Q heads + 1 KV head: wq/wk/wv split column-wise, wo split
row-wise; each core produces a partial (2048, 4096) output that the host sums
(the all-reduce of row-parallel wo).

Per-core kernel, bf16 matmul operands (1 cyc/row at any free size, half the
DMA/SBUF/DVE traffic of f32; PSUM accumulation stays fp32):
  A) QKV projections over per-chunk x tiles: an interleaved K||V pass tracks
     the x DMA stream, then Q0..Q3 passes re-read the resident chunks while
     RoPE / V-transpose work drains on ACT+DVE+PE underneath. Weight DMAs
     ride the (otherwise idle) GPSIMD queue so the x stream owns the sync
     queue's DMA engine.
  B/C) software-pipelined: the attention streams for query block qb carry
     the output projection of block qb-1 inside them. Per (head h, qb):
     S^T matmuls + exp (ACT, scale folded) + in-place 128x128 0/1 triangle
     mask on the diagonal (DVE) + row-sum accumulation (split GPSIMD/DVE)
     + lagged AV drains, with the wo-projection pair groups of (qb-1, qc=h)
     and the previous head's deferred tail drains / D-reciprocal chain
     interleaved between score matmuls. Diagonal blocks shrink their moving
     free dim to the causally-visible suffix. D is broadcast across
     partitions via a ones-matrix matmul, inverted with the fast custom-DVE
     reciprocal, and applied to O^T on DVE.
"""

import numpy as np

import concourse.bacc as bacc
import concourse.tile as tile
from concourse import mybir
from concourse.bass_utils import run_bass_kernel_spmd

F32 = mybir.dt.float32
F32R = mybir.dt.float32r
BF16 = mybir.dt.bfloat16

DIM = 4096
SEQ = 2048
HEAD_DIM = 128
N_CORES = 8
QH = 4              # q heads per core
QS = QH * HEAD_DIM  # 512: wq column slice per core
NKT = DIM // 128    # 32 contraction tiles
NSB = SEQ // 512    # 4 sequence blocks
SCALE = 1.0 / float(np.sqrt(HEAD_DIM))
LAG = 4             # AV matmuls trail the score stream by LAG blocks


def build_nc():
    nc = bacc.Bacc(trn_type="TRN2")

    xT = nc.declare_dram_parameter("xT", [DIM, SEQ], BF16, isOutput=False)
    wq = nc.declare_dram_parameter("wq", [DIM, QS], BF16, isOutput=False)
    wk = nc.declare_dram_parameter("wk", [DIM, HEAD_DIM], BF16, isOutput=False)
    wv = nc.declare_dram_parameter("wv", [DIM, HEAD_DIM], BF16, isOutput=False)
    wo = nc.declare_dram_parameter("wo", [QS, DIM], BF16, isOutput=False)
    cosT = nc.declare_dram_parameter("cosT", [HEAD_DIM, SEQ], BF16, isOutput=False)
    sinTs = nc.declare_dram_parameter("sinTs", [HEAD_DIM, SEQ], BF16, isOutput=False)
    tri = nc.declare_dram_parameter("tri", [128, 128], BF16, isOutput=False)
    ident = nc.declare_dram_parameter("ident", [128, 128], BF16, isOutput=False)
    ones_mat = nc.declare_dram_parameter("ones_mat", [128, 128], F32R, isOutput=False)
    out = nc.declare_dram_parameter("out", [SEQ, DIM], BF16, isOutput=True)

    with tile.TileContext(nc) as tc:
        with (
            tc.tile_pool(name="persist", bufs=1) as persist,
            tc.tile_pool(name="resid", bufs=1) as resid,
        ):
            tri_sb = persist.tile([128, 128], BF16)
            nc.scalar.dma_start(out=tri_sb, in_=tri[:, :])
            ident_sb = persist.tile([128, 128], BF16)
            nc.scalar.dma_start(out=ident_sb, in_=ident[:, :])
            ones_sb = persist.tile([128, 128], F32R)
            nc.scalar.dma_start(out=ones_sb, in_=ones_mat[:, :])

            # resident activations; qT split per head so phase B's first
            # score matmuls only wait on their own head's RoPE
            qTs = [resid.tile([128, SEQ], BF16, name=f"qT{h}")
                   for h in range(QH)]
            kT = resid.tile([128, SEQ], BF16)              # K^T (d, s)
            vN = resid.tile([128, SEQ // 128, 128], BF16)  # V natural (k, d)

            # ---------------- Phase A: projections + RoPE ----------------
            with (
                tc.tile_pool(name="wpool", bufs=1) as wpool,
                tc.tile_pool(name="xpool", bufs=2) as xpool,
                tc.tile_pool(name="cspool", bufs=2) as cspool,
                tc.tile_pool(name="ropetmp", bufs=2) as ropetmp,
                tc.tile_pool(name="vtb", bufs=2) as vtb,
                tc.tile_pool(name="psA", bufs=1, space="PSUM") as psA,
                tc.tile_pool(name="psVT", bufs=2, space="PSUM") as psVT,
            ):
                # per-chunk weight tiles: a matmul only waits on the one DMA
                # that feeds its chunk, not the whole weight load. Weights
                # ride the GPSIMD DMA queue, x owns the sync queue.
                wk_cs = [wpool.tile([128, 16, HEAD_DIM], BF16, name=f"wk{c}")
                         for c in range(2)]
                wv_cs = [wpool.tile([128, 16, HEAD_DIM], BF16, name=f"wv{c}")
                         for c in range(2)]
                wq_cs = [wpool.tile([128, 8, QS], BF16, name=f"wq{c}")
                         for c in range(4)]
                wq_r = wq.rearrange("(t p) m -> p t m", p=128)
                wk_r = wk.rearrange("(t p) m -> p t m", p=128)
                wv_r = wv.rearrange("(t p) m -> p t m", p=128)
                xT_r = xT.rearrange("(t p) s -> p t s", p=128)

                def xchunks(sb):
                    xs = []
                    for g in range(8):
                        xg = xpool.tile([128, 4, 512], BF16, tag=f"xb{g}",
                                        name=f"xb{g}")
                        nc.sync.dma_start(
                            out=xg,
                            in_=xT_r[:, g * 4:(g + 1) * 4,
                                     sb * 512:(sb + 1) * 512],
                        )
                        xs.append(xg)
                    return xs

                nc.gpsimd.dma_start(out=wk_cs[0], in_=wk_r[:, 0:16, :])
                nc.gpsimd.dma_start(out=wv_cs[0], in_=wv_r[:, 0:16, :])
                xb = xchunks(0)
                nc.gpsimd.dma_start(out=wk_cs[1], in_=wk_r[:, 16:32, :])
                nc.gpsimd.dma_start(out=wv_cs[1], in_=wv_r[:, 16:32, :])
                cos_t = cspool.tile([128, 512], BF16, tag="cos")
                nc.scalar.dma_start(out=cos_t, in_=cosT[:, 0:512])
                sin_t = cspool.tile([128, 512], BF16, tag="sin")
                nc.scalar.dma_start(out=sin_t, in_=sinTs[:, 0:512])
                for c in range(4):
                    nc.gpsimd.dma_start(
                        out=wq_cs[c], in_=wq_r[:, c * 8:(c + 1) * 8, :]
                    )

                def rope(dst, src_ps, cos_t, sin_t):
                    # ACT copies evict PSUM fast and produce the straight and
                    # half-rotated views (partition-shifted reads are only
                    # legal on ACT); DVE runs the bf16 mul/mul/add at 2x rate.
                    v = ropetmp.tile([128, 512], BF16, tag="v", name="v")
                    vr = ropetmp.tile([128, 512], BF16, tag="vr", name="vr")
                    nc.scalar.copy(v, src_ps)
                    nc.scalar.copy(vr[0:64, :], src_ps[64:128, :])
                    nc.scalar.copy(vr[64:128, :], src_ps[0:64, :])
                    t = ropetmp.tile([128, 512], BF16, tag="t", name="t")
                    u = ropetmp.tile([128, 512], BF16, tag="u", name="u")
                    nc.vector.tensor_mul(t, v, cos_t)
                    nc.vector.tensor_mul(u, vr, sin_t)
                    nc.vector.tensor_add(dst, t, u)

                for sb in range(NSB):
                    ss = slice(sb * 512, (sb + 1) * 512)
                    if sb > 0:
                        xb = xchunks(sb)
                        cos_t = cspool.tile([128, 512], BF16, tag="cos")
                        nc.scalar.dma_start(out=cos_t, in_=cosT[:, ss])
                        sin_t = cspool.tile([128, 512], BF16, tag="sin")
                        nc.scalar.dma_start(out=sin_t, in_=sinTs[:, ss])

                    # interleaved K||V pass (each x chunk feeds both, so the
                    # PE tracks the x DMA stream); rope-K and the V evict
                    # then run on ACT/DVE under the Q passes
                    k_ps = psA.tile([128, 512], F32, tag="kps", name="kps")
                    v_ps = psA.tile([128, 512], F32, tag="vps", name="vps")
                    for kt in range(NKT):
                        nc.tensor.matmul(
                            k_ps, wk_cs[kt // 16][:, kt % 16, :],
                            xb[kt // 4][:, kt % 4, :],
                            start=(kt == 0), stop=(kt == NKT - 1),
                        )
                        nc.tensor.matmul(
                            v_ps, wv_cs[kt // 16][:, kt % 16, :],
                            xb[kt // 4][:, kt % 4, :],
                            start=(kt == 0), stop=(kt == NKT - 1),
                        )
                    rope(kT[:, ss], k_ps, cos_t, sin_t)
                    vt_sb = vtb.tile([128, 512], BF16, tag="vt", name="vt")
                    nc.scalar.copy(vt_sb, v_ps)

                    for h in range(QH):
                        q_ps = psA.tile([128, 512], F32, tag=f"qps{h}",
                                        name=f"qps{h}")
                        for kt in range(NKT):
                            nc.tensor.matmul(
                                q_ps,
                                wq_cs[kt // 8][:, kt % 8,
                                               h * 128:(h + 1) * 128],
                                xb[kt // 4][:, kt % 4, :],
                                start=(kt == 0), stop=(kt == NKT - 1),
                            )
                        if h == 0:
                            # PE transposes of V^T -> V, slotted between Q
                            # passes so they never wait on the ACT evict
                            for j in range(4):
                                vt_ps = psVT.tile([128, 128], BF16, tag="vtp",
                                                  name="vtp")
                                nc.tensor.transpose(
                                    vt_ps, vt_sb[:, j * 128:(j + 1) * 128],
                                    ident_sb,
                                )
                                nc.scalar.copy(vN[:, sb * 4 + j, :], vt_ps)
                        rope(qTs[h][:, ss], q_ps, cos_t, sin_t)

            # ------- Phase B/C: attention with pipelined out projection -------
            with (
                tc.tile_pool(name="wopool", bufs=1) as wopool,
                tc.tile_pool(name="expp", bufs=24) as expp,
                tc.tile_pool(name="esum", bufs=2) as esum,
                tc.tile_pool(name="rdp", bufs=2) as rdp,
                tc.tile_pool(name="otp", bufs=2) as otp,
                tc.tile_pool(name="stg", bufs=2) as stg,
                # declaration order maps pools onto the banks phase A frees
                # first (k/v accumulators release during the Q passes; the
                # q accumulators only after their trailing RoPE)
                tc.tile_pool(name="psOT", bufs=2, space="PSUM") as psOT,
                tc.tile_pool(name="psS", bufs=2, space="PSUM") as psS,
                tc.tile_pool(name="psD", bufs=2, space="PSUM") as psD,
                tc.tile_pool(name="psC", bufs=2, space="PSUM") as psC,
            ):
                wo_sbs = [wopool.tile([128, DIM], BF16, name=f"wo{h}")
                          for h in range(QH)]
                wo_r = wo.rearrange("(h p) n -> p h n", p=128)
                for h in range(QH):
                    nc.gpsimd.dma_start(out=wo_sbs[h], in_=wo_r[:, h, :])

                ot_store = {}   # (qb, h) -> normalized O^T tile
                deferred = []   # closures: tail drains + D chains, issued
                                # under later PE work

                def flush(n=None):
                    k = len(deferred) if n is None else min(n, len(deferred))
                    for _ in range(k):
                        deferred.pop(0)()

                def c_items(qb, qc, last=False):
                    """Output-projection work of query block qb, row stripe
                    qc: 8 accumulation groups + evicts + 2 half-stripe DMAs,
                    as a list of closures."""
                    stg_t = stg.tile([128, DIM], BF16, tag="stg", name="stg")
                    items = []

                    def group(nb, evict_dve):
                        def go():
                            o_ps = psC.tile([128, 512], F32, tag="ops",
                                            name="ops")
                            for h in range(QH):
                                nc.tensor.matmul(
                                    o_ps,
                                    ot_store[(qb, h)][:, qc * 128:
                                                      (qc + 1) * 128],
                                    wo_sbs[h][:, nb * 512:(nb + 1) * 512],
                                    start=(h == 0), stop=(h == QH - 1),
                                )
                            dst = stg_t[:, nb * 512:(nb + 1) * 512]
                            if evict_dve:
                                nc.vector.tensor_copy(dst, o_ps)
                            else:
                                nc.scalar.copy(dst, o_ps)
                            if last:
                                nc.sync.dma_start(
                                    out=out[qb * 512 + qc * 128:
                                            qb * 512 + (qc + 1) * 128,
                                            nb * 512:(nb + 1) * 512],
                                    in_=dst,
                                )
                        return go

                    def dma(lo, hi):
                        def go():
                            nc.sync.dma_start(
                                out=out[qb * 512 + qc * 128:
                                        qb * 512 + (qc + 1) * 128,
                                        lo * 512:hi * 512],
                                in_=stg_t[:, lo * 512:hi * 512],
                            )
                        return go

                    for nb in range(8):
                        # during-stream evicts lean on DVE (ACT is running
                        # the exp stream); the trailing block alternates
                        items.append(group(nb, evict_dve=(nb < 5) if not last
                                           else nb % 2 == 0))
                        if not last:
                            if nb == 3:
                                items.append(dma(0, 4))
                            elif nb == 7:
                                items.append(dma(4, 8))
                    return items

                for qb in range(NSB):
                    qs = slice(qb * 512, (qb + 1) * 512)
                    n_kb = 4 * qb + 4
                    for h in range(QH):
                        citems = c_items(qb - 1, h) if qb > 0 else []
                        c_done = 0
                        ot_ps = psOT.tile([128, 512], F32, tag="otps",
                                          name="otps")
                        # row-sum accumulators: [tile, start offset or None]
                        acc_a = [esum.tile([128, 512], F32R, tag="esa",
                                           name="esa"), None]
                        acc_b = [esum.tile([128, 512], F32R, tag="esb",
                                           name="esb"), None]
                        ess = [None] * n_kb
                        offs = [max(0, kb - 4 * qb) * 128
                                for kb in range(n_kb)]

                        def drain(kb, ot_ps=ot_ps, ess=ess, n_kb=n_kb,
                                  offs=offs):
                            o = offs[kb]
                            nc.tensor.matmul(
                                ot_ps[:, o:], vN[:, kb, :], ess[kb][:, o:],
                                start=(kb == 0), stop=(kb == n_kb - 1),
                            )

                        for kb in range(n_kb):
                            off = offs[kb]
                            s_ps = psS.tile([128, 512], F32, tag="sps",
                                            name="sps")
                            nc.tensor.matmul(
                                s_ps[:, off:],
                                kT[:, kb * 128:(kb + 1) * 128],
                                qTs[h][:, qb * 512 + off:(qb + 1) * 512],
                                start=True, stop=True,
                            )
                            es = expp.tile([128, 512], BF16, tag="es",
                                           name="es")
                            nc.scalar.activation(
                                es[:, off:], s_ps[:, off:],
                                mybir.ActivationFunctionType.Exp,
                                scale=SCALE,
                            )
                            if kb - 4 * qb >= 0:
                                # in-place 0/1 lower-triangle mask on the
                                # 128-wide diagonal sub-block
                                nc.vector.tensor_mul(
                                    es[:, off:off + 128],
                                    es[:, off:off + 128], tri_sb,
                                )
                            ess[kb] = es
                            # row-sum accumulation split GPSIMD/DVE (GPSIMD
                            # tensor ops run ~1.2us each, so it only gets a
                            # third of the stream)
                            if kb % 3 == 0:
                                eng, acc = nc.gpsimd, acc_a
                            else:
                                eng, acc = nc.vector, acc_b
                            if acc[1] is None:
                                eng.tensor_copy(acc[0][:, off:], es[:, off:])
                                acc[1] = off
                            else:
                                eng.tensor_add(acc[0][:, off:],
                                               acc[0][:, off:], es[:, off:])
                            if kb >= LAG:
                                drain(kb - LAG)
                            # previous head's deferred chain, then this
                            # head's share of the qb-1 projection work
                            if kb == 0:
                                flush(2)
                            elif kb == 1:
                                flush(2)
                            elif kb == 2:
                                flush()
                            elif citems:
                                want = (len(citems) * (kb - 2)) // (n_kb - 3)
                                while c_done < want:
                                    citems[c_done]()
                                    c_done += 1
                        while c_done < len(citems):
                            citems[c_done]()
                            c_done += 1
                        for kb in range(max(0, n_kb - LAG), n_kb):
                            deferred.append(
                                lambda kb=kb, drain=drain: drain(kb)
                            )

                        def dchain(qb=qb, h=h, ot_ps=ot_ps, acc_a=acc_a,
                                   acc_b=acc_b):
                            # D broadcast across partitions (each output row
                            # of ones^T @ es_sum is the key-dim column sum),
                            # fast reciprocal, O^T scale
                            d_ps = psD.tile([128, 512], F32, tag="dps",
                                            name="dps")
                            accs = [a for a in (acc_a, acc_b)
                                    if a[1] is not None]
                            for i, (t, o) in enumerate(accs):
                                nc.tensor.matmul(
                                    d_ps[:, o:], ones_sb, t[:, o:],
                                    start=(i == 0), stop=(i == len(accs) - 1),
                                )
                            rd = rdp.tile([128, 512], F32, tag="rd",
                                          name="rd")
                            nc.vector.reciprocal_approx_fast(out=rd, in_=d_ps)
                            ot = otp.tile([128, 512], BF16, tag=f"ot{h}",
                                          name=f"ot{h}")
                            nc.vector.tensor_mul(ot, ot_ps, rd)
                            ot_store[(qb, h)] = ot

                        deferred.append(dchain)

                # trailing projection of the last query block
                flush()
                for qc in range(QH):
                    for it in c_items(NSB - 1, qc, last=True):
                        it()
    nc.finalize()
    return nc


_NC_CACHE = {}


def _get_nc():
    if "nc" not in _NC_CACHE:
        _NC_CACHE["nc"] = build_nc()
    return _NC_CACHE["nc"]


def _host_prep(x, cos, sin, mask, wq, wk, wv, wo):
    import ml_dtypes

    bf16 = ml_dtypes.bfloat16
    xT = np.ascontiguousarray(x[0].T).astype(bf16)
    cosT = np.ascontiguousarray(cos[:, 0, :].T).astype(bf16)
    sinT = sin[:, 0, :].T.astype(np.float32)
    sinTs = np.ascontiguousarray(
        np.concatenate([-sinT[:64], sinT[64:]], axis=0)
    ).astype(bf16)
    rr = np.arange(128, dtype=np.int64)[:, None]
    cc = np.arange(128, dtype=np.int64)[None, :]
    tri = (rr <= cc).astype(np.float32).astype(bf16)
    ident = np.eye(128).astype(bf16)
    ones_mat = np.ones((128, 128), dtype=np.float32)

    in_maps = []
    for i in range(N_CORES):
        in_maps.append({
            "xT": xT,
            "wq": np.ascontiguousarray(wq[:, i * QS:(i + 1) * QS]).astype(bf16),
            "wk": np.ascontiguousarray(wk[:, i * 128:(i + 1) * 128]).astype(bf16),
            "wv": np.ascontiguousarray(wv[:, i * 128:(i + 1) * 128]).astype(bf16),
            "wo": np.ascontiguousarray(wo[i * QS:(i + 1) * QS, :]).astype(bf16),
            "cosT": cosT,
            "sinTs": sinTs,
            "tri": tri,
            "ident": ident,
            "ones_mat": ones_mat,
        })
    return in_maps


def kernel(x, cos, sin, mask, wq, wk, wv, wo, _trace=False, _trace_kwargs=None):
    nc = _get_nc()
    in_maps = _host_prep(x, cos, sin, mask, wq, wk, wv, wo)
    res = run_bass_kernel_spmd(
        nc, in_maps, list(range(N_CORES)), trace=_trace,
        **(_trace_kwargs or {}),
    )
    partials = [res.results[i]["out"] for i in range(N_CORES)]
    full = np.sum(
        np.stack([p.astype(np.float32) for p in partials], axis=0),
        axis=0, dtype=np.float64,
    )
    out = full.astype(np.float32)[None, :, :]
    if _trace:
        return out, res
    return out


# revision 13
# speedup vs baseline: 1.5311x; 1.0059x over previous
"""GQA attention (SEQ=2048, DIM=4096, 32 Q heads / 8 KV heads, head_dim=128),
tensor-parallel over heads across 8 NeuronCores.

Each core owns 4 Q heads + 1 KV head: wq/wk/wv split column-wise, wo split
row-wise; each core produces a partial (2048, 4096) output that the host sums
(the all-reduce of row-parallel wo).

Per-core kernel, bf16 matmul operands (1 cyc/row at any free size, half the
DMA/SBUF/DVE traffic of f32; PSUM accumulation stays fp32):
  A) QKV projections over per-chunk x tiles: an interleaved K||V pass tracks
     the x DMA stream, then Q0..Q3 passes re-read the resident chunks while
     RoPE / V-transpose work drains on ACT+DVE+PE underneath. Weight DMAs
     ride the (otherwise idle) GPSIMD queue so the x stream owns the sync
     queue's DMA engine.
  B/C) software-pipelined: the attention streams for query block qb carry
     the output projection of block qb-1 inside them. Per (head h, qb):
     S^T matmuls + exp (ACT, scale folded) + in-place 128x128 0/1 triangle
     mask on the diagonal (DVE) + row-sum accumulation (split GPSIMD/DVE)
     + lagged AV drains, with the wo-projection pair groups of (qb-1, qc=h)
     and the previous head's deferred tail drains / D-reciprocal chain
     interleaved between score matmuls. Diagonal blocks shrink their moving
     free dim to the causally-visible suffix. D is broadcast across
     partitions via a ones-matrix matmul, inverted with the fast custom-DVE
     reciprocal, and applied to O^T on DVE.
"""

import numpy as np

import concourse.bacc as bacc
import concourse.tile as tile
from concourse import mybir
from concourse.bass_utils import run_bass_kernel_spmd

F32 = mybir.dt.float32
F32R = mybir.dt.float32r
BF16 = mybir.dt.bfloat16

DIM = 4096
SEQ = 2048
HEAD_DIM = 128
N_CORES = 8
QH = 4              # q heads per core
QS = QH * HEAD_DIM  # 512: wq column slice per core
NKT = DIM // 128    # 32 contraction tiles
NSB = SEQ // 512    # 4 sequence blocks
SCALE = 1.0 / float(np.sqrt(HEAD_DIM))
LAG = 4             # AV matmuls trail the score stream by LAG blocks


def build_nc():
    nc = bacc.Bacc(trn_type="TRN2")

    xT = nc.declare_dram_parameter("xT", [DIM, SEQ], BF16, isOutput=False)
    wq = nc.declare_dram_parameter("wq", [DIM, QS], BF16, isOutput=False)
    wk = nc.declare_dram_parameter("wk", [DIM, HEAD_DIM], BF16, isOutput=False)
    wv = nc.declare_dram_parameter("wv", [DIM, HEAD_DIM], BF16, isOutput=False)
    wo = nc.declare_dram_parameter("wo", [QS, DIM], BF16, isOutput=False)
    cosT = nc.declare_dram_parameter("cosT", [HEAD_DIM, SEQ], BF16, isOutput=False)
    sinTs = nc.declare_dram_parameter("sinTs", [HEAD_DIM, SEQ], BF16, isOutput=False)
    tri = nc.declare_dram_parameter("tri", [128, 128], BF16, isOutput=False)
    ident = nc.declare_dram_parameter("ident", [128, 128], BF16, isOutput=False)
    ones_mat = nc.declare_dram_parameter("ones_mat", [128, 128], F32R, isOutput=False)
    out = nc.declare_dram_parameter("out", [SEQ, DIM], BF16, isOutput=True)

    with tile.TileContext(nc) as tc:
        with (
            tc.tile_pool(name="persist", bufs=1) as persist,
            tc.tile_pool(name="resid", bufs=1) as resid,
        ):
            tri_sb = persist.tile([128, 128], BF16)
            nc.scalar.dma_start(out=tri_sb, in_=tri[:, :])
            ident_sb = persist.tile([128, 128], BF16)
            nc.scalar.dma_start(out=ident_sb, in_=ident[:, :])
            ones_sb = persist.tile([128, 128], F32R)
            nc.scalar.dma_start(out=ones_sb, in_=ones_mat[:, :])

            # resident activations; qT split per head so phase B's first
            # score matmuls only wait on their own head's RoPE
            qTs = [resid.tile([128, SEQ], BF16, name=f"qT{h}")
                   for h in range(QH)]
            kT = resid.tile([128, SEQ], BF16)              # K^T (d, s)
            vN = resid.tile([128, SEQ // 128, 128], BF16)  # V natural (k, d)

            # ---------------- Phase A: projections + RoPE ----------------
            with (
                tc.tile_pool(name="wpool", bufs=1) as wpool,
                tc.tile_pool(name="xpool", bufs=2) as xpool,
                tc.tile_pool(name="cspool", bufs=2) as cspool,
                tc.tile_pool(name="ropetmp", bufs=2) as ropetmp,
                tc.tile_pool(name="vtb", bufs=2) as vtb,
                tc.tile_pool(name="psA", bufs=1, space="PSUM") as psA,
                tc.tile_pool(name="psVT", bufs=2, space="PSUM") as psVT,
            ):
                # per-chunk weight tiles: a matmul only waits on the one DMA
                # that feeds its chunk, not the whole weight load. Weights
                # ride the scalar HW-DGE queue, x owns the sync queue.
                wk_cs = [wpool.tile([128, 16, HEAD_DIM], BF16, name=f"wk{c}")
                         for c in range(2)]
                wv_cs = [wpool.tile([128, 16, HEAD_DIM], BF16, name=f"wv{c}")
                         for c in range(2)]
                wq_cs = [wpool.tile([128, 8, QS], BF16, name=f"wq{c}")
                         for c in range(4)]
                wq_r = wq.rearrange("(t p) m -> p t m", p=128)
                wk_r = wk.rearrange("(t p) m -> p t m", p=128)
                wv_r = wv.rearrange("(t p) m -> p t m", p=128)
                xT_r = xT.rearrange("(t p) s -> p t s", p=128)

                def xchunks(sb):
                    xs = []
                    for g in range(8):
                        xg = xpool.tile([128, 4, 512], BF16, tag=f"xb{g}",
                                        name=f"xb{g}")
                        nc.sync.dma_start(
                            out=xg,
                            in_=xT_r[:, g * 4:(g + 1) * 4,
                                     sb * 512:(sb + 1) * 512],
                        )
                        xs.append(xg)
                    return xs

                nc.scalar.dma_start(out=wk_cs[0], in_=wk_r[:, 0:16, :])
                nc.scalar.dma_start(out=wv_cs[0], in_=wv_r[:, 0:16, :])
                xb = xchunks(0)
                nc.scalar.dma_start(out=wk_cs[1], in_=wk_r[:, 16:32, :])
                nc.scalar.dma_start(out=wv_cs[1], in_=wv_r[:, 16:32, :])
                cos_t = cspool.tile([128, 512], BF16, tag="cos")
                nc.scalar.dma_start(out=cos_t, in_=cosT[:, 0:512])
                sin_t = cspool.tile([128, 512], BF16, tag="sin")
                nc.scalar.dma_start(out=sin_t, in_=sinTs[:, 0:512])
                for c in range(4):
                    nc.scalar.dma_start(
                        out=wq_cs[c], in_=wq_r[:, c * 8:(c + 1) * 8, :]
                    )

                def rope(dst, src_ps, cos_t, sin_t):
                    # ACT copies evict PSUM fast and produce the straight and
                    # half-rotated views (partition-shifted reads are only
                    # legal on ACT); DVE runs the bf16 mul/mul/add at 2x rate.
                    v = ropetmp.tile([128, 512], BF16, tag="v", name="v")
                    vr = ropetmp.tile([128, 512], BF16, tag="vr", name="vr")
                    nc.scalar.copy(v, src_ps)
                    nc.scalar.copy(vr[0:64, :], src_ps[64:128, :])
                    nc.scalar.copy(vr[64:128, :], src_ps[0:64, :])
                    t = ropetmp.tile([128, 512], BF16, tag="t", name="t")
                    u = ropetmp.tile([128, 512], BF16, tag="u", name="u")
                    nc.vector.tensor_mul(t, v, cos_t)
                    nc.vector.tensor_mul(u, vr, sin_t)
                    nc.vector.tensor_add(dst, t, u)

                for sb in range(NSB):
                    ss = slice(sb * 512, (sb + 1) * 512)
                    if sb > 0:
                        xb = xchunks(sb)
                        cos_t = cspool.tile([128, 512], BF16, tag="cos")
                        nc.scalar.dma_start(out=cos_t, in_=cosT[:, ss])
                        sin_t = cspool.tile([128, 512], BF16, tag="sin")
                        nc.scalar.dma_start(out=sin_t, in_=sinTs[:, ss])

                    # interleaved K||V pass (each x chunk feeds both, so the
                    # PE tracks the x DMA stream); rope-K and the V evict
                    # then run on ACT/DVE under the Q passes
                    k_ps = psA.tile([128, 512], F32, tag="kps", name="kps")
                    v_ps = psA.tile([128, 512], F32, tag="vps", name="vps")
                    for kt in range(NKT):
                        nc.tensor.matmul(
                            k_ps, wk_cs[kt // 16][:, kt % 16, :],
                            xb[kt // 4][:, kt % 4, :],
                            start=(kt == 0), stop=(kt == NKT - 1),
                        )
                        nc.tensor.matmul(
                            v_ps, wv_cs[kt // 16][:, kt % 16, :],
                            xb[kt // 4][:, kt % 4, :],
                            start=(kt == 0), stop=(kt == NKT - 1),
                        )
                    rope(kT[:, ss], k_ps, cos_t, sin_t)
                    vt_sb = vtb.tile([128, 512], BF16, tag="vt", name="vt")
                    nc.scalar.copy(vt_sb, v_ps)

                    for h in range(QH):
                        q_ps = psA.tile([128, 512], F32, tag=f"qps{h}",
                                        name=f"qps{h}")
                        for kt in range(NKT):
                            nc.tensor.matmul(
                                q_ps,
                                wq_cs[kt // 8][:, kt % 8,
                                               h * 128:(h + 1) * 128],
                                xb[kt // 4][:, kt % 4, :],
                                start=(kt == 0), stop=(kt == NKT - 1),
                            )
                        if h == 0:
                            # PE transposes of V^T -> V, slotted between Q
                            # passes so they never wait on the ACT evict
                            for j in range(4):
                                vt_ps = psVT.tile([128, 128], BF16, tag="vtp",
                                                  name="vtp")
                                nc.tensor.transpose(
                                    vt_ps, vt_sb[:, j * 128:(j + 1) * 128],
                                    ident_sb,
                                )
                                nc.scalar.copy(vN[:, sb * 4 + j, :], vt_ps)
                        rope(qTs[h][:, ss], q_ps, cos_t, sin_t)

            # ------- Phase B/C: attention with pipelined out projection -------
            with (
                tc.tile_pool(name="wopool", bufs=1) as wopool,
                tc.tile_pool(name="expp", bufs=24) as expp,
                tc.tile_pool(name="esum", bufs=2) as esum,
                tc.tile_pool(name="rdp", bufs=2) as rdp,
                tc.tile_pool(name="otp", bufs=2) as otp,
                tc.tile_pool(name="stg", bufs=2) as stg,
                # declaration order maps pools onto the banks phase A frees
                # first (k/v accumulators release during the Q passes; the
                # q accumulators only after their trailing RoPE)
                tc.tile_pool(name="psOT", bufs=2, space="PSUM") as psOT,
                tc.tile_pool(name="psS", bufs=2, space="PSUM") as psS,
                tc.tile_pool(name="psD", bufs=2, space="PSUM") as psD,
                tc.tile_pool(name="psC", bufs=2, space="PSUM") as psC,
            ):
                wo_sbs = [wopool.tile([128, DIM], BF16, name=f"wo{h}")
                          for h in range(QH)]
                wo_r = wo.rearrange("(h p) n -> p h n", p=128)
                for h in range(QH):
                    nc.scalar.dma_start(out=wo_sbs[h], in_=wo_r[:, h, :])

                ot_store = {}   # (qb, h) -> normalized O^T tile
                deferred = []   # closures: tail drains + D chains, issued
                                # under later PE work

                def flush(n=None):
                    k = len(deferred) if n is None else min(n, len(deferred))
                    for _ in range(k):
                        deferred.pop(0)()

                def c_items(qb, qc, last=False):
                    """Output-projection work of query block qb, row stripe
                    qc: 8 accumulation groups + evicts + 2 half-stripe DMAs,
                    as a list of closures."""
                    stg_t = stg.tile([128, DIM], BF16, tag="stg", name="stg")
                    items = []

                    def group(nb, evict_dve):
                        def go():
                            o_ps = psC.tile([128, 512], F32, tag="ops",
                                            name="ops")
                            for h in range(QH):
                                nc.tensor.matmul(
                                    o_ps,
                                    ot_store[(qb, h)][:, qc * 128:
                                                      (qc + 1) * 128],
                                    wo_sbs[h][:, nb * 512:(nb + 1) * 512],
                                    start=(h == 0), stop=(h == QH - 1),
                                )
                            dst = stg_t[:, nb * 512:(nb + 1) * 512]
                            if evict_dve:
                                nc.vector.tensor_copy(dst, o_ps)
                            else:
                                nc.scalar.copy(dst, o_ps)
                            if last:
                                nc.sync.dma_start(
                                    out=out[qb * 512 + qc * 128:
                                            qb * 512 + (qc + 1) * 128,
                                            nb * 512:(nb + 1) * 512],
                                    in_=dst,
                                )
                        return go

                    def dma(lo, hi):
                        def go():
                            nc.sync.dma_start(
                                out=out[qb * 512 + qc * 128:
                                        qb * 512 + (qc + 1) * 128,
                                        lo * 512:hi * 512],
                                in_=stg_t[:, lo * 512:hi * 512],
                            )
                        return go

                    for nb in range(8):
                        # during-stream evicts lean on DVE (ACT is running
                        # the exp stream); the trailing block alternates
                        items.append(group(nb, evict_dve=(nb < 5) if not last
                                           else nb % 2 == 0))
                        if not last:
                            if nb == 3:
                                items.append(dma(0, 4))
                            elif nb == 7:
                                items.append(dma(4, 8))
                    return items

                for qb in range(NSB):
                    qs = slice(qb * 512, (qb + 1) * 512)
                    n_kb = 4 * qb + 4
                    for h in range(QH):
                        citems = c_items(qb - 1, h) if qb > 0 else []
                        c_done = 0
                        ot_ps = psOT.tile([128, 512], F32, tag="otps",
                                          name="otps")
                        # row-sum accumulators: [tile, start offset or None]
                        acc_a = [esum.tile([128, 512], F32R, tag="esa",
                                           name="esa"), None]
                        acc_b = [esum.tile([128, 512], F32R, tag="esb",
                                           name="esb"), None]
                        ess = [None] * n_kb
                        offs = [max(0, kb - 4 * qb) * 128
                                for kb in range(n_kb)]

                        def drain(kb, ot_ps=ot_ps, ess=ess, n_kb=n_kb,
                                  offs=offs):
                            o = offs[kb]
                            nc.tensor.matmul(
                                ot_ps[:, o:], vN[:, kb, :], ess[kb][:, o:],
                                start=(kb == 0), stop=(kb == n_kb - 1),
                            )

                        for kb in range(n_kb):
                            off = offs[kb]
                            s_ps = psS.tile([128, 512], F32, tag="sps",
                                            name="sps")
                            nc.tensor.matmul(
                                s_ps[:, off:],
                                kT[:, kb * 128:(kb + 1) * 128],
                                qTs[h][:, qb * 512 + off:(qb + 1) * 512],
                                start=True, stop=True,
                            )
                            es = expp.tile([128, 512], BF16, tag="es",
                                           name="es")
                            nc.scalar.activation(
                                es[:, off:], s_ps[:, off:],
                                mybir.ActivationFunctionType.Exp,
                                scale=SCALE,
                            )
                            if kb - 4 * qb >= 0:
                                # in-place 0/1 lower-triangle mask on the
                                # 128-wide diagonal sub-block
                                nc.vector.tensor_mul(
                                    es[:, off:off + 128],
                                    es[:, off:off + 128], tri_sb,
                                )
                            ess[kb] = es
                            # row-sum accumulation split GPSIMD/DVE (GPSIMD
                            # tensor ops run ~1.2us each, so it only gets a
                            # third of the stream)
                            if kb % 3 == 0:
                                eng, acc = nc.gpsimd, acc_a
                            else:
                                eng, acc = nc.vector, acc_b
                            if acc[1] is None:
                                eng.tensor_copy(acc[0][:, off:], es[:, off:])
                                acc[1] = off
                            else:
                                eng.tensor_add(acc[0][:, off:],
                                               acc[0][:, off:], es[:, off:])
                            if kb >= LAG:
                                drain(kb - LAG)
                            # previous head's deferred chain, then this
                            # head's share of the qb-1 projection work
                            if kb == 0:
                                flush(2)
                            elif kb == 1:
                                flush(2)
                            elif kb == 2:
                                flush()
                            elif citems:
                                want = (len(citems) * (kb - 2)) // (n_kb - 3)
                                while c_done < want:
                                    citems[c_done]()
                                    c_done += 1
                        while c_done < len(citems):
                            citems[c_done]()
                            c_done += 1
                        for kb in range(max(0, n_kb - LAG), n_kb):
                            deferred.append(
                                lambda kb=kb, drain=drain: drain(kb)
                            )

                        def dchain(qb=qb, h=h, ot_ps=ot_ps, acc_a=acc_a,
                                   acc_b=acc_b):
                            # D broadcast across partitions (each output row
                            # of ones^T @ es_sum is the key-dim column sum),
                            # fast reciprocal, O^T scale
                            d_ps = psD.tile([128, 512], F32, tag="dps",
                                            name="dps")
                            accs = [a for a in (acc_a, acc_b)
                                    if a[1] is not None]
                            for i, (t, o) in enumerate(accs):
                                nc.tensor.matmul(
                                    d_ps[:, o:], ones_sb, t[:, o:],
                                    start=(i == 0), stop=(i == len(accs) - 1),
                                )
                            rd = rdp.tile([128, 512], F32, tag="rd",
                                          name="rd")
                            nc.vector.reciprocal_approx_fast(out=rd, in_=d_ps)
                            ot = otp.tile([128, 512], BF16, tag=f"ot{h}",
                                          name=f"ot{h}")
                            nc.vector.tensor_mul(ot, ot_ps, rd)
                            ot_store[(qb, h)] = ot

                        deferred.append(dchain)

                # trailing projection of the last query block
                flush()
                for qc in range(QH):
                    for it in c_items(NSB - 1, qc, last=True):
                        it()
    nc.finalize()
    return nc


_NC_CACHE = {}


def _get_nc():
    if "nc" not in _NC_CACHE:
        _NC_CACHE["nc"] = build_nc()
    return _NC_CACHE["nc"]


def _host_prep(x, cos, sin, mask, wq, wk, wv, wo):
    import ml_dtypes

    bf16 = ml_dtypes.bfloat16
    xT = np.ascontiguousarray(x[0].T).astype(bf16)
    cosT = np.ascontiguousarray(cos[:, 0, :].T).astype(bf16)
    sinT = sin[:, 0, :].T.astype(np.float32)
    sinTs = np.ascontiguousarray(
        np.concatenate([-sinT[:64], sinT[64:]], axis=0)
    ).astype(bf16)
    rr = np.arange(128, dtype=np.int64)[:, None]
    cc = np.arange(128, dtype=np.int64)[None, :]
    tri = (rr <= cc).astype(np.float32).astype(bf16)
    ident = np.eye(128).astype(bf16)
    ones_mat = np.ones((128, 128), dtype=np.float32)

    in_maps = []
    for i in range(N_CORES):
        in_maps.append({
            "xT": xT,
            "wq": np.ascontiguousarray(wq[:, i * QS:(i + 1) * QS]).astype(bf16),
            "wk": np.ascontiguousarray(wk[:, i * 128:(i + 1) * 128]).astype(bf16),
            "wv": np.ascontiguousarray(wv[:, i * 128:(i + 1) * 128]).astype(bf16),
            "wo": np.ascontiguousarray(wo[i * QS:(i + 1) * QS, :]).astype(bf16),
            "cosT": cosT,
            "sinTs": sinTs,
            "tri": tri,
            "ident": ident,
            "ones_mat": ones_mat,
        })
    return in_maps


def kernel(x, cos, sin, mask, wq, wk, wv, wo, _trace=False, _trace_kwargs=None):
    nc = _get_nc()
    in_maps = _host_prep(x, cos, sin, mask, wq, wk, wv, wo)
    res = run_bass_kernel_spmd(
        nc, in_maps, list(range(N_CORES)), trace=_trace,
        **(_trace_kwargs or {}),
    )
    partials = [res.results[i]["out"] for i in range(N_CORES)]
    full = np.sum(
        np.stack([p.astype(np.float32) for p in partials], axis=0),
        axis=0, dtype=np.float64,
    )
    out = full.astype(np.float32)[None, :, :]
    if _trace:
        return out, res
    return out


# revision 14
# speedup vs baseline: 1.5605x; 1.0192x over previous
"""GQA attention (SEQ=2048, DIM=4096, 32 Q heads / 8 KV heads, head_dim=128),
tensor-parallel over heads across 8 NeuronCores.

Each core owns 4 Q heads + 1 KV head: wq/wk/wv split column-wise, wo split
row-wise; each core produces a partial (2048, 4096) output that the host sums
(the all-reduce of row-parallel wo).

Per-core kernel, bf16 matmul operands (1 cyc/row at any free size, half the
DMA/SBUF/DVE traffic of f32; PSUM accumulation stays fp32):
  A) QKV projections over per-chunk x tiles: an interleaved K||V pass tracks
     the x DMA stream, then Q0..Q3 passes re-read the resident chunks while
     RoPE / V-transpose work drains on ACT+DVE+PE underneath. Weight DMAs
     ride the (otherwise idle) GPSIMD queue so the x stream owns the sync
     queue's DMA engine.
  B/C) software-pipelined: the attention streams for query block qb carry
     the output projection of block qb-1 inside them. Per (head h, qb):
     S^T matmuls + exp (ACT, scale folded) + in-place 128x128 0/1 triangle
     mask on the diagonal (DVE) + row-sum accumulation (split GPSIMD/DVE)
     + lagged AV drains, with the wo-projection pair groups of (qb-1, qc=h)
     and the previous head's deferred tail drains / D-reciprocal chain
     interleaved between score matmuls. Diagonal blocks shrink their moving
     free dim to the causally-visible suffix. D is broadcast across
     partitions via a ones-matrix matmul, inverted with the fast custom-DVE
     reciprocal, and applied to O^T on DVE.
"""

import numpy as np

import concourse.bacc as bacc
import concourse.tile as tile
from concourse import mybir
from concourse.bass_utils import run_bass_kernel_spmd

F32 = mybir.dt.float32
F32R = mybir.dt.float32r
BF16 = mybir.dt.bfloat16

DIM = 4096
SEQ = 2048
HEAD_DIM = 128
N_CORES = 8
QH = 4              # q heads per core
QS = QH * HEAD_DIM  # 512: wq column slice per core
NKT = DIM // 128    # 32 contraction tiles
NSB = SEQ // 512    # 4 sequence blocks
SCALE = 1.0 / float(np.sqrt(HEAD_DIM))
LAG = 4             # AV matmuls trail the score stream by LAG blocks


def build_nc():
    nc = bacc.Bacc(trn_type="TRN2")

    # all big operands are pre-shuffled on the host into partition-major
    # layouts so every DMA moves 4-8 KiB contiguous lines per partition
    xS = nc.declare_dram_parameter("xS", [128, NSB, NKT, 512], BF16, isOutput=False)
    wqS = nc.declare_dram_parameter("wqS", [128, NKT * QS], BF16, isOutput=False)
    wkS = nc.declare_dram_parameter("wkS", [128, NKT * HEAD_DIM], BF16, isOutput=False)
    wvS = nc.declare_dram_parameter("wvS", [128, NKT * HEAD_DIM], BF16, isOutput=False)
    woS = nc.declare_dram_parameter("woS", [128, QH * DIM], BF16, isOutput=False)
    cosT = nc.declare_dram_parameter("cosT", [HEAD_DIM, SEQ], BF16, isOutput=False)
    sinTs = nc.declare_dram_parameter("sinTs", [HEAD_DIM, SEQ], BF16, isOutput=False)
    tri = nc.declare_dram_parameter("tri", [128, 128], BF16, isOutput=False)
    ident = nc.declare_dram_parameter("ident", [128, 128], BF16, isOutput=False)
    ones_mat = nc.declare_dram_parameter("ones_mat", [128, 128], F32R, isOutput=False)
    out = nc.declare_dram_parameter("out", [SEQ, DIM], BF16, isOutput=True)

    with tile.TileContext(nc) as tc:
        with (
            tc.tile_pool(name="persist", bufs=1) as persist,
            tc.tile_pool(name="resid", bufs=1) as resid,
        ):
            tri_sb = persist.tile([128, 128], BF16)
            ident_sb = persist.tile([128, 128], BF16)
            ones_sb = persist.tile([128, 128], F32R)

            # resident activations; qT split per head so phase B's first
            # score matmuls only wait on their own head's RoPE
            qTs = [resid.tile([128, SEQ], BF16, name=f"qT{h}")
                   for h in range(QH)]
            kT = resid.tile([128, SEQ], BF16)              # K^T (d, s)
            vN = resid.tile([128, SEQ // 128, 128], BF16)  # V natural (k, d)

            # ---------------- Phase A: projections + RoPE ----------------
            with (
                tc.tile_pool(name="wpool", bufs=1) as wpool,
                tc.tile_pool(name="xpool", bufs=2) as xpool,
                tc.tile_pool(name="cspool", bufs=2) as cspool,
                tc.tile_pool(name="ropetmp", bufs=2) as ropetmp,
                tc.tile_pool(name="vtb", bufs=2) as vtb,
                tc.tile_pool(name="psA", bufs=1, space="PSUM") as psA,
                tc.tile_pool(name="psVT", bufs=2, space="PSUM") as psVT,
            ):
                # per-chunk weight tiles: a matmul only waits on the one DMA
                # that feeds its chunk, not the whole weight load. Weights
                # ride the scalar HW-DGE queue, x owns the sync queue.
                wk_cs = [wpool.tile([128, 16 * HEAD_DIM], BF16, name=f"wk{c}")
                         for c in range(2)]
                wv_cs = [wpool.tile([128, 16 * HEAD_DIM], BF16, name=f"wv{c}")
                         for c in range(2)]
                wq_cs = [wpool.tile([128, 8 * QS], BF16, name=f"wq{c}")
                         for c in range(4)]

                def xchunks(sb):
                    xs = []
                    for g in range(8):
                        xg = xpool.tile([128, 4, 512], BF16, tag=f"xb{g}",
                                        name=f"xb{g}")
                        nc.sync.dma_start(out=xg, in_=xS[:, sb, g * 4:(g + 1) * 4, :])
                        xs.append(xg)
                    return xs

                nc.scalar.dma_start(out=wk_cs[0], in_=wkS[:, 0:2048])
                nc.scalar.dma_start(out=wv_cs[0], in_=wvS[:, 0:2048])
                xb = xchunks(0)
                nc.scalar.dma_start(out=wk_cs[1], in_=wkS[:, 2048:4096])
                nc.scalar.dma_start(out=wv_cs[1], in_=wvS[:, 2048:4096])
                cos_t = cspool.tile([128, 512], BF16, tag="cos")
                nc.scalar.dma_start(out=cos_t, in_=cosT[:, 0:512])
                sin_t = cspool.tile([128, 512], BF16, tag="sin")
                nc.scalar.dma_start(out=sin_t, in_=sinTs[:, 0:512])
                for c in range(4):
                    nc.scalar.dma_start(
                        out=wq_cs[c], in_=wqS[:, c * 4096:(c + 1) * 4096]
                    )

                # small constants ride behind the critical weight DMAs
                nc.scalar.dma_start(out=ident_sb, in_=ident[:, :])
                nc.scalar.dma_start(out=tri_sb, in_=tri[:, :])
                nc.scalar.dma_start(out=ones_sb, in_=ones_mat[:, :])

                def rope(dst, src_ps, cos_t, sin_t):
                    # ACT copies evict PSUM fast and produce the straight and
                    # half-rotated views (partition-shifted reads are only
                    # legal on ACT); DVE runs the bf16 mul/mul/add at 2x rate.
                    v = ropetmp.tile([128, 512], BF16, tag="v", name="v")
                    vr = ropetmp.tile([128, 512], BF16, tag="vr", name="vr")
                    nc.scalar.copy(v, src_ps)
                    nc.scalar.copy(vr[0:64, :], src_ps[64:128, :])
                    nc.scalar.copy(vr[64:128, :], src_ps[0:64, :])
                    t = ropetmp.tile([128, 512], BF16, tag="t", name="t")
                    u = ropetmp.tile([128, 512], BF16, tag="u", name="u")
                    nc.vector.tensor_mul(t, v, cos_t)
                    nc.vector.tensor_mul(u, vr, sin_t)
                    nc.vector.tensor_add(dst, t, u)

                for sb in range(NSB):
                    ss = slice(sb * 512, (sb + 1) * 512)
                    if sb > 0:
                        xb = xchunks(sb)
                        cos_t = cspool.tile([128, 512], BF16, tag="cos")
                        nc.scalar.dma_start(out=cos_t, in_=cosT[:, ss])
                        sin_t = cspool.tile([128, 512], BF16, tag="sin")
                        nc.scalar.dma_start(out=sin_t, in_=sinTs[:, ss])

                    # interleaved K||V pass (each x chunk feeds both, so the
                    # PE tracks the x DMA stream); rope-K and the V evict
                    # then run on ACT/DVE under the Q passes
                    k_ps = psA.tile([128, 512], F32, tag="kps", name="kps")
                    v_ps = psA.tile([128, 512], F32, tag="vps", name="vps")
                    for kt in range(NKT):
                        nc.tensor.matmul(
                            k_ps,
                            wk_cs[kt // 16][:, (kt % 16) * 128:
                                            (kt % 16 + 1) * 128],
                            xb[kt // 4][:, kt % 4, :],
                            start=(kt == 0), stop=(kt == NKT - 1),
                        )
                        nc.tensor.matmul(
                            v_ps,
                            wv_cs[kt // 16][:, (kt % 16) * 128:
                                            (kt % 16 + 1) * 128],
                            xb[kt // 4][:, kt % 4, :],
                            start=(kt == 0), stop=(kt == NKT - 1),
                        )
                    rope(kT[:, ss], k_ps, cos_t, sin_t)
                    vt_sb = vtb.tile([128, 512], BF16, tag="vt", name="vt")
                    nc.scalar.copy(vt_sb, v_ps)

                    for h in range(QH):
                        q_ps = psA.tile([128, 512], F32, tag=f"qps{h}",
                                        name=f"qps{h}")
                        for kt in range(NKT):
                            nc.tensor.matmul(
                                q_ps,
                                wq_cs[kt // 8][:, (kt % 8) * 512 + h * 128:
                                               (kt % 8) * 512 + (h + 1) * 128],
                                xb[kt // 4][:, kt % 4, :],
                                start=(kt == 0), stop=(kt == NKT - 1),
                            )
                        if h == 0:
                            # PE transposes of V^T -> V, slotted between Q
                            # passes so they never wait on the ACT evict
                            for j in range(4):
                                vt_ps = psVT.tile([128, 128], BF16, tag="vtp",
                                                  name="vtp")
                                nc.tensor.transpose(
                                    vt_ps, vt_sb[:, j * 128:(j + 1) * 128],
                                    ident_sb,
                                )
                                nc.scalar.copy(vN[:, sb * 4 + j, :], vt_ps)
                        rope(qTs[h][:, ss], q_ps, cos_t, sin_t)

            # ------- Phase B/C: attention with pipelined out projection -------
            with (
                tc.tile_pool(name="wopool", bufs=1) as wopool,
                tc.tile_pool(name="expp", bufs=24) as expp,
                tc.tile_pool(name="esum", bufs=2) as esum,
                tc.tile_pool(name="rdp", bufs=2) as rdp,
                tc.tile_pool(name="otp", bufs=2) as otp,
                tc.tile_pool(name="stg", bufs=2) as stg,
                # declaration order maps pools onto the banks phase A frees
                # first (k/v accumulators release during the Q passes; the
                # q accumulators only after their trailing RoPE)
                tc.tile_pool(name="psOT", bufs=2, space="PSUM") as psOT,
                tc.tile_pool(name="psS", bufs=2, space="PSUM") as psS,
                tc.tile_pool(name="psD", bufs=2, space="PSUM") as psD,
                tc.tile_pool(name="psC", bufs=2, space="PSUM") as psC,
            ):
                wo_sbs = [wopool.tile([128, DIM], BF16, name=f"wo{h}")
                          for h in range(QH)]
                for h in range(QH):
                    nc.scalar.dma_start(
                        out=wo_sbs[h], in_=woS[:, h * DIM:(h + 1) * DIM]
                    )

                ot_store = {}   # (qb, h) -> normalized O^T tile
                deferred = []   # closures: tail drains + D chains, issued
                                # under later PE work

                def flush(n=None):
                    k = len(deferred) if n is None else min(n, len(deferred))
                    for _ in range(k):
                        deferred.pop(0)()

                def c_items(qb, qc, last=False):
                    """Output-projection work of query block qb, row stripe
                    qc: 8 accumulation groups + evicts + 2 half-stripe DMAs,
                    as a list of closures."""
                    stg_t = stg.tile([128, DIM], BF16, tag="stg", name="stg")
                    items = []

                    def group(nb, evict_dve):
                        def go():
                            o_ps = psC.tile([128, 512], F32, tag="ops",
                                            name="ops")
                            for h in range(QH):
                                nc.tensor.matmul(
                                    o_ps,
                                    ot_store[(qb, h)][:, qc * 128:
                                                      (qc + 1) * 128],
                                    wo_sbs[h][:, nb * 512:(nb + 1) * 512],
                                    start=(h == 0), stop=(h == QH - 1),
                                )
                            dst = stg_t[:, nb * 512:(nb + 1) * 512]
                            if evict_dve:
                                nc.vector.tensor_copy(dst, o_ps)
                            else:
                                nc.scalar.copy(dst, o_ps)
                            if last:
                                nc.sync.dma_start(
                                    out=out[qb * 512 + qc * 128:
                                            qb * 512 + (qc + 1) * 128,
                                            nb * 512:(nb + 1) * 512],
                                    in_=dst,
                                )
                        return go

                    def dma(lo, hi):
                        def go():
                            nc.sync.dma_start(
                                out=out[qb * 512 + qc * 128:
                                        qb * 512 + (qc + 1) * 128,
                                        lo * 512:hi * 512],
                                in_=stg_t[:, lo * 512:hi * 512],
                            )
                        return go

                    for nb in range(8):
                        # during-stream evicts lean on DVE (ACT is running
                        # the exp stream); the trailing block alternates
                        items.append(group(nb, evict_dve=(nb < 5) if not last
                                           else nb % 2 == 0))
                        if not last:
                            if nb == 3:
                                items.append(dma(0, 4))
                            elif nb == 7:
                                items.append(dma(4, 8))
                    return items

                for qb in range(NSB):
                    qs = slice(qb * 512, (qb + 1) * 512)
                    n_kb = 4 * qb + 4
                    for h in range(QH):
                        citems = c_items(qb - 1, h) if qb > 0 else []
                        c_done = 0
                        ot_ps = psOT.tile([128, 512], F32, tag="otps",
                                          name="otps")
                        # row-sum accumulators: [tile, start offset or None]
                        acc_a = [esum.tile([128, 512], F32R, tag="esa",
                                           name="esa"), None]
                        acc_b = [esum.tile([128, 512], F32R, tag="esb",
                                           name="esb"), None]
                        ess = [None] * n_kb
                        offs = [max(0, kb - 4 * qb) * 128
                                for kb in range(n_kb)]

                        def drain(kb, ot_ps=ot_ps, ess=ess, n_kb=n_kb,
                                  offs=offs):
                            o = offs[kb]
                            nc.tensor.matmul(
                                ot_ps[:, o:], vN[:, kb, :], ess[kb][:, o:],
                                start=(kb == 0), stop=(kb == n_kb - 1),
                            )

                        for kb in range(n_kb):
                            off = offs[kb]
                            s_ps = psS.tile([128, 512], F32, tag="sps",
                                            name="sps")
                            nc.tensor.matmul(
                                s_ps[:, off:],
                                kT[:, kb * 128:(kb + 1) * 128],
                                qTs[h][:, qb * 512 + off:(qb + 1) * 512],
                                start=True, stop=True,
                            )
                            es = expp.tile([128, 512], BF16, tag="es",
                                           name="es")
                            nc.scalar.activation(
                                es[:, off:], s_ps[:, off:],
                                mybir.ActivationFunctionType.Exp,
                                scale=SCALE,
                            )
                            if kb - 4 * qb >= 0:
                                # in-place 0/1 lower-triangle mask on the
                                # 128-wide diagonal sub-block
                                nc.vector.tensor_mul(
                                    es[:, off:off + 128],
                                    es[:, off:off + 128], tri_sb,
                                )
                            ess[kb] = es
                            # row-sum accumulation split GPSIMD/DVE (GPSIMD
                            # tensor ops run ~1.2us each, so it only gets a
                            # third of the stream)
                            if kb % 3 == 0:
                                eng, acc = nc.gpsimd, acc_a
                            else:
                                eng, acc = nc.vector, acc_b
                            if acc[1] is None:
                                eng.tensor_copy(acc[0][:, off:], es[:, off:])
                                acc[1] = off
                            else:
                                eng.tensor_add(acc[0][:, off:],
                                               acc[0][:, off:], es[:, off:])
                            if kb >= LAG:
                                drain(kb - LAG)
                            # previous head's deferred chain, then this
                            # head's share of the qb-1 projection work
                            if kb == 0:
                                flush(2)
                            elif kb == 1:
                                flush(2)
                            elif kb == 2:
                                flush()
                            elif citems:
                                want = (len(citems) * (kb - 2)) // (n_kb - 3)
                                while c_done < want:
                                    citems[c_done]()
                                    c_done += 1
                        while c_done < len(citems):
                            citems[c_done]()
                            c_done += 1
                        for kb in range(max(0, n_kb - LAG), n_kb):
                            deferred.append(
                                lambda kb=kb, drain=drain: drain(kb)
                            )

                        def dchain(qb=qb, h=h, ot_ps=ot_ps, acc_a=acc_a,
                                   acc_b=acc_b):
                            # D broadcast across partitions (each output row
                            # of ones^T @ es_sum is the key-dim column sum),
                            # fast reciprocal, O^T scale
                            d_ps = psD.tile([128, 512], F32, tag="dps",
                                            name="dps")
                            accs = [a for a in (acc_a, acc_b)
                                    if a[1] is not None]
                            for i, (t, o) in enumerate(accs):
                                nc.tensor.matmul(
                                    d_ps[:, o:], ones_sb, t[:, o:],
                                    start=(i == 0), stop=(i == len(accs) - 1),
                                )
                            rd = rdp.tile([128, 512], F32, tag="rd",
                                          name="rd")
                            nc.vector.reciprocal_approx_fast(out=rd, in_=d_ps)
                            ot = otp.tile([128, 512], BF16, tag=f"ot{h}",
                                          name=f"ot{h}")
                            nc.vector.tensor_mul(ot, ot_ps, rd)
                            ot_store[(qb, h)] = ot

                        deferred.append(dchain)

                # trailing projection of the last query block
                flush()
                for qc in range(QH):
                    for it in c_items(NSB - 1, qc, last=True):
                        it()
    nc.finalize()
    return nc


_NC_CACHE = {}


def _get_nc():
    if "nc" not in _NC_CACHE:
        _NC_CACHE["nc"] = build_nc()
    return _NC_CACHE["nc"]


def _host_prep(x, cos, sin, mask, wq, wk, wv, wo):
    import ml_dtypes

    bf16 = ml_dtypes.bfloat16
    # partition-major shuffles: index [p, ...] with contraction tile t so
    # every DMA line is 4-8 KiB contiguous
    xS = np.ascontiguousarray(
        x[0].astype(bf16)                    # (S, D) = (sb*512+s, t*128+p)
        .reshape(NSB, 512, NKT, 128)
        .transpose(3, 0, 2, 1)               # (p, sb, t, s)
    )
    cosT = np.ascontiguousarray(cos[:, 0, :].T).astype(bf16)
    sinT = sin[:, 0, :].T.astype(np.float32)
    sinTs = np.ascontiguousarray(
        np.concatenate([-sinT[:64], sinT[64:]], axis=0)
    ).astype(bf16)
    rr = np.arange(128, dtype=np.int64)[:, None]
    cc = np.arange(128, dtype=np.int64)[None, :]
    tri = (rr <= cc).astype(np.float32).astype(bf16)
    ident = np.eye(128).astype(bf16)
    ones_mat = np.ones((128, 128), dtype=np.float32)

    def wshuf(w):
        # (t*128+p, m) -> (p, t*M+m)
        t = w.shape[0] // 128
        return np.ascontiguousarray(
            w.astype(bf16).reshape(t, 128, -1).transpose(1, 0, 2)
            .reshape(128, -1)
        )

    in_maps = []
    for i in range(N_CORES):
        in_maps.append({
            "xS": xS,
            "wqS": wshuf(wq[:, i * QS:(i + 1) * QS]),
            "wkS": wshuf(wk[:, i * 128:(i + 1) * 128]),
            "wvS": wshuf(wv[:, i * 128:(i + 1) * 128]),
            "woS": wshuf(wo[i * QS:(i + 1) * QS, :]),
            "cosT": cosT,
            "sinTs": sinTs,
            "tri": tri,
            "ident": ident,
            "ones_mat": ones_mat,
        })
    return in_maps


def kernel(x, cos, sin, mask, wq, wk, wv, wo, _trace=False, _trace_kwargs=None):
    nc = _get_nc()
    in_maps = _host_prep(x, cos, sin, mask, wq, wk, wv, wo)
    res = run_bass_kernel_spmd(
        nc, in_maps, list(range(N_CORES)), trace=_trace,
        **(_trace_kwargs or {}),
    )
    partials = [res.results[i]["out"] for i in range(N_CORES)]
    full = np.sum(
        np.stack([p.astype(np.float32) for p in partials], axis=0),
        axis=0, dtype=np.float64,
    )
    out = full.astype(np.float32)[None, :, :]
    if _trace:
        return out, res
    return out


# revision 16
# speedup vs baseline: 1.6098x; 1.0316x over previous
"""GQA attention (SEQ=2048, DIM=4096, 32 Q heads / 8 KV heads, head_dim=128),
tensor-parallel over heads across 8 NeuronCores.

Each core owns 4 Q heads + 1 KV head: wq/wk/wv split column-wise, wo split
row-wise; each core produces a partial (2048, 4096) output that the host sums
(the all-reduce of row-parallel wo).

Per-core kernel, bf16 matmul operands (1 cyc/row at any free size, half the
DMA/SBUF/DVE traffic of f32; PSUM accumulation stays fp32):
  A) QKV projections over per-chunk x tiles: an interleaved K||V pass tracks
     the x DMA stream, then Q0..Q3 passes re-read the resident chunks while
     RoPE / V-transpose work drains on ACT+DVE+PE underneath. Weight DMAs
     ride the (otherwise idle) GPSIMD queue so the x stream owns the sync
     queue's DMA engine.
  B/C) software-pipelined: the attention streams for query block qb carry
     the output projection of block qb-1 inside them. Per (head h, qb):
     S^T matmuls + exp (ACT, scale folded) + in-place 128x128 0/1 triangle
     mask on the diagonal (DVE) + row-sum accumulation (split GPSIMD/DVE)
     + lagged AV drains, with the wo-projection pair groups of (qb-1, qc=h)
     and the previous head's deferred tail drains / D-reciprocal chain
     interleaved between score matmuls. Diagonal blocks shrink their moving
     free dim to the causally-visible suffix. D is broadcast across
     partitions via a ones-matrix matmul, inverted with the fast custom-DVE
     reciprocal, and applied to O^T on DVE.
"""

import numpy as np

import concourse.bacc as bacc
import concourse.tile as tile
from concourse import mybir
from concourse.bass_utils import run_bass_kernel_spmd

F32 = mybir.dt.float32
F32R = mybir.dt.float32r
BF16 = mybir.dt.bfloat16

DIM = 4096
SEQ = 2048
HEAD_DIM = 128
N_CORES = 8
QH = 4              # q heads per core
QS = QH * HEAD_DIM  # 512: wq column slice per core
NKT = DIM // 128    # 32 contraction tiles
NSB = SEQ // 512    # 4 sequence blocks
SCALE = 1.0 / float(np.sqrt(HEAD_DIM))
LAG = 4             # AV matmuls trail the score stream by LAG blocks


def build_nc():
    nc = bacc.Bacc(trn_type="TRN2")

    # all big operands are pre-shuffled on the host into partition-major
    # layouts so every DMA moves 4-8 KiB contiguous lines per partition
    xS = nc.declare_dram_parameter("xS", [128, NSB, NKT, 512], BF16, isOutput=False)
    wqS = nc.declare_dram_parameter("wqS", [128, NKT * QS], BF16, isOutput=False)
    wkS = nc.declare_dram_parameter("wkS", [128, NKT * HEAD_DIM], BF16, isOutput=False)
    wvS = nc.declare_dram_parameter("wvS", [128, NKT * HEAD_DIM], BF16, isOutput=False)
    woS = nc.declare_dram_parameter("woS", [128, QH * DIM], BF16, isOutput=False)
    cosT = nc.declare_dram_parameter("cosT", [HEAD_DIM, SEQ], BF16, isOutput=False)
    sinTs = nc.declare_dram_parameter("sinTs", [HEAD_DIM, SEQ], BF16, isOutput=False)
    tri = nc.declare_dram_parameter("tri", [128, 128], BF16, isOutput=False)
    ident = nc.declare_dram_parameter("ident", [128, 128], BF16, isOutput=False)
    ones_mat = nc.declare_dram_parameter("ones_mat", [128, 128], F32R, isOutput=False)
    out = nc.declare_dram_parameter("out", [SEQ, DIM], BF16, isOutput=True)

    with tile.TileContext(nc) as tc:
        with (
            tc.tile_pool(name="persist", bufs=1) as persist,
            tc.tile_pool(name="resid", bufs=1) as resid,
        ):
            tri_sb = persist.tile([128, 128], BF16)
            ident_sb = persist.tile([128, 128], BF16)
            ones_sb = persist.tile([128, 128], F32R)

            # resident activations; qT split per head so phase B's first
            # score matmuls only wait on their own head's RoPE
            qTs = [resid.tile([128, SEQ], BF16, name=f"qT{h}")
                   for h in range(QH)]
            kT = resid.tile([128, SEQ], BF16)              # K^T (d, s)
            vN = resid.tile([128, SEQ // 128, 128], BF16)  # V natural (k, d)

            # ---------------- Phase A: projections + RoPE ----------------
            with (
                tc.tile_pool(name="wpool", bufs=1) as wpool,
                tc.tile_pool(name="xpool", bufs=2) as xpool,
                tc.tile_pool(name="cspool", bufs=2) as cspool,
                tc.tile_pool(name="ropetmp", bufs=2) as ropetmp,
                tc.tile_pool(name="vtb", bufs=2) as vtb,
                tc.tile_pool(name="psA", bufs=1, space="PSUM") as psA,
                tc.tile_pool(name="psVT", bufs=2, space="PSUM") as psVT,
            ):
                # per-chunk weight tiles: a matmul only waits on the one DMA
                # that feeds its chunk, not the whole weight load. Weights
                # ride the scalar HW-DGE queue, x owns the sync queue.
                wk_cs = [wpool.tile([128, 16 * HEAD_DIM], BF16, name=f"wk{c}")
                         for c in range(2)]
                wv_cs = [wpool.tile([128, 16 * HEAD_DIM], BF16, name=f"wv{c}")
                         for c in range(2)]
                wq_cs = [wpool.tile([128, 8 * QS], BF16, name=f"wq{c}")
                         for c in range(4)]

                def xchunks(sb):
                    xs = []
                    for g in range(8):
                        xg = xpool.tile([128, 4, 512], BF16, tag=f"xb{g}",
                                        name=f"xb{g}")
                        nc.sync.dma_start(out=xg, in_=xS[:, sb, g * 4:(g + 1) * 4, :])
                        xs.append(xg)
                    return xs

                nc.scalar.dma_start(out=wk_cs[0], in_=wkS[:, 0:2048])
                nc.scalar.dma_start(out=wv_cs[0], in_=wvS[:, 0:2048])
                xb = xchunks(0)
                nc.scalar.dma_start(out=wq_cs[0], in_=wqS[:, 0:4096])
                cos_t = cspool.tile([128, 512], BF16, tag="cos")
                nc.scalar.dma_start(out=cos_t, in_=cosT[:, 0:512])
                sin_t = cspool.tile([128, 512], BF16, tag="sin")
                nc.scalar.dma_start(out=sin_t, in_=sinTs[:, 0:512])
                nc.scalar.dma_start(out=wq_cs[1], in_=wqS[:, 4096:8192])
                nc.scalar.dma_start(out=wk_cs[1], in_=wkS[:, 2048:4096])
                nc.scalar.dma_start(out=wv_cs[1], in_=wvS[:, 2048:4096])
                nc.scalar.dma_start(out=wq_cs[2], in_=wqS[:, 8192:12288])
                nc.scalar.dma_start(out=wq_cs[3], in_=wqS[:, 12288:16384])

                # small constants ride behind the critical weight DMAs
                nc.scalar.dma_start(out=ident_sb, in_=ident[:, :])
                nc.scalar.dma_start(out=tri_sb, in_=tri[:, :])
                nc.scalar.dma_start(out=ones_sb, in_=ones_mat[:, :])

                def rope(dst, src_ps, cos_t, sin_t):
                    # ACT copies evict PSUM fast and produce the straight and
                    # half-rotated views (partition-shifted reads are only
                    # legal on ACT); DVE runs the bf16 mul/mul/add at 2x rate.
                    v = ropetmp.tile([128, 512], BF16, tag="v", name="v")
                    vr = ropetmp.tile([128, 512], BF16, tag="vr", name="vr")
                    nc.scalar.copy(v, src_ps)
                    nc.scalar.copy(vr[0:64, :], src_ps[64:128, :])
                    nc.scalar.copy(vr[64:128, :], src_ps[0:64, :])
                    t = ropetmp.tile([128, 512], BF16, tag="t", name="t")
                    u = ropetmp.tile([128, 512], BF16, tag="u", name="u")
                    nc.vector.tensor_mul(t, v, cos_t)
                    nc.vector.tensor_mul(u, vr, sin_t)
                    nc.vector.tensor_add(dst, t, u)

                for sb in range(NSB):
                    ss = slice(sb * 512, (sb + 1) * 512)
                    if sb > 0:
                        xb = xchunks(sb)
                        cos_t = cspool.tile([128, 512], BF16, tag="cos")
                        nc.scalar.dma_start(out=cos_t, in_=cosT[:, ss])
                        sin_t = cspool.tile([128, 512], BF16, tag="sin")
                        nc.scalar.dma_start(out=sin_t, in_=sinTs[:, ss])

                    # interleaved K||V(||Q0 for the cold-start block)
                    # pass: the PE consumes each x chunk at the DMA delivery
                    # rate; rope-K and the V evict then run on ACT/DVE under
                    # the remaining Q passes
                    def qmm(q_ps, h, kt):
                        nc.tensor.matmul(
                            q_ps,
                            wq_cs[kt // 8][:, (kt % 8) * 512 + h * 128:
                                           (kt % 8) * 512 + (h + 1) * 128],
                            xb[kt // 4][:, kt % 4, :],
                            start=(kt == 0), stop=(kt == NKT - 1),
                        )

                    k_ps = psA.tile([128, 512], F32, tag="kps", name="kps")
                    v_ps = psA.tile([128, 512], F32, tag="vps", name="vps")
                    q_ps0 = psA.tile([128, 512], F32, tag="qps0", name="qps0")
                    for kt in range(NKT):
                        nc.tensor.matmul(
                            k_ps,
                            wk_cs[kt // 16][:, (kt % 16) * 128:
                                            (kt % 16 + 1) * 128],
                            xb[kt // 4][:, kt % 4, :],
                            start=(kt == 0), stop=(kt == NKT - 1),
                        )
                        nc.tensor.matmul(
                            v_ps,
                            wv_cs[kt // 16][:, (kt % 16) * 128:
                                            (kt % 16 + 1) * 128],
                            xb[kt // 4][:, kt % 4, :],
                            start=(kt == 0), stop=(kt == NKT - 1),
                        )
                        if sb == 0:
                            qmm(q_ps0, 0, kt)
                    rope(kT[:, ss], k_ps, cos_t, sin_t)
                    vt_sb = vtb.tile([128, 512], BF16, tag="vt", name="vt")
                    nc.scalar.copy(vt_sb, v_ps)
                    if sb == 0:
                        rope(qTs[0][:, ss], q_ps0, cos_t, sin_t)

                    for h in range(QH):
                        if h == 0:
                            if sb == 0:
                                continue
                            q_ps = q_ps0
                        else:
                            q_ps = psA.tile([128, 512], F32, tag=f"qps{h}",
                                            name=f"qps{h}")
                        for kt in range(NKT):
                            qmm(q_ps, h, kt)
                        if h == 1:
                            # PE transposes of V^T -> V, slotted between Q
                            # passes so they never wait on the ACT evict
                            for j in range(4):
                                vt_ps = psVT.tile([128, 128], BF16, tag="vtp",
                                                  name="vtp")
                                nc.tensor.transpose(
                                    vt_ps, vt_sb[:, j * 128:(j + 1) * 128],
                                    ident_sb,
                                )
                                nc.scalar.copy(vN[:, sb * 4 + j, :], vt_ps)
                        rope(qTs[h][:, ss], q_ps, cos_t, sin_t)

            # ------- Phase B/C: attention with pipelined out projection -------
            with (
                tc.tile_pool(name="wopool", bufs=1) as wopool,
                tc.tile_pool(name="expp", bufs=24) as expp,
                tc.tile_pool(name="esum", bufs=2) as esum,
                tc.tile_pool(name="rdp", bufs=2) as rdp,
                tc.tile_pool(name="otp", bufs=2) as otp,
                tc.tile_pool(name="stg", bufs=2) as stg,
                # declaration order maps pools onto the banks phase A frees
                # first (k/v accumulators release during the Q passes; the
                # q accumulators only after their trailing RoPE)
                tc.tile_pool(name="psOT", bufs=2, space="PSUM") as psOT,
                tc.tile_pool(name="psS", bufs=2, space="PSUM") as psS,
                tc.tile_pool(name="psD", bufs=2, space="PSUM") as psD,
                tc.tile_pool(name="psC", bufs=2, space="PSUM") as psC,
            ):
                wo_sbs = [wopool.tile([128, DIM], BF16, name=f"wo{h}")
                          for h in range(QH)]
                for h in range(QH):
                    nc.scalar.dma_start(
                        out=wo_sbs[h], in_=woS[:, h * DIM:(h + 1) * DIM]
                    )

                ot_store = {}   # (qb, h) -> normalized O^T tile
                deferred = []   # closures: tail drains + D chains, issued
                                # under later PE work

                def flush(n=None):
                    k = len(deferred) if n is None else min(n, len(deferred))
                    for _ in range(k):
                        deferred.pop(0)()

                def c_items(qb, qc, last=False):
                    """Output-projection work of query block qb, row stripe
                    qc: 8 accumulation groups + evicts + 2 half-stripe DMAs,
                    as a list of closures."""
                    stg_t = stg.tile([128, DIM], BF16, tag="stg", name="stg")
                    items = []

                    def group(nb, evict_dve):
                        def go():
                            o_ps = psC.tile([128, 512], F32, tag="ops",
                                            name="ops")
                            for h in range(QH):
                                nc.tensor.matmul(
                                    o_ps,
                                    ot_store[(qb, h)][:, qc * 128:
                                                      (qc + 1) * 128],
                                    wo_sbs[h][:, nb * 512:(nb + 1) * 512],
                                    start=(h == 0), stop=(h == QH - 1),
                                )
                            dst = stg_t[:, nb * 512:(nb + 1) * 512]
                            if evict_dve:
                                nc.vector.tensor_copy(dst, o_ps)
                            else:
                                nc.scalar.copy(dst, o_ps)
                            if last:
                                nc.sync.dma_start(
                                    out=out[qb * 512 + qc * 128:
                                            qb * 512 + (qc + 1) * 128,
                                            nb * 512:(nb + 1) * 512],
                                    in_=dst,
                                )
                        return go

                    def dma(lo, hi):
                        def go():
                            nc.sync.dma_start(
                                out=out[qb * 512 + qc * 128:
                                        qb * 512 + (qc + 1) * 128,
                                        lo * 512:hi * 512],
                                in_=stg_t[:, lo * 512:hi * 512],
                            )
                        return go

                    for nb in range(8):
                        # during-stream evicts lean on DVE (ACT is running
                        # the exp stream); the trailing block alternates
                        items.append(group(nb, evict_dve=(nb < 5) if not last
                                           else nb % 2 == 0))
                        if not last:
                            if nb == 3:
                                items.append(dma(0, 4))
                            elif nb == 7:
                                items.append(dma(4, 8))
                    return items

                for qb in range(NSB):
                    qs = slice(qb * 512, (qb + 1) * 512)
                    n_kb = 4 * qb + 4
                    for h in range(QH):
                        citems = c_items(qb - 1, h) if qb > 0 else []
                        c_done = 0
                        ot_ps = psOT.tile([128, 512], F32, tag="otps",
                                          name="otps")
                        # row-sum accumulators: [tile, start offset or None,
                        # engine] -- one fed by GPSIMD, one by DVE
                        accs = [
                            [esum.tile([128, 512], F32R, tag="esa",
                                       name="esa"), None, nc.gpsimd],
                            [esum.tile([128, 512], F32R, tag="esb",
                                       name="esb"), None, nc.vector],
                        ]
                        nacc = [0]
                        pend = [None]

                        def acc_push(t, off):
                            a = accs[nacc[0] % 2]
                            nacc[0] += 1
                            if a[1] is None:
                                a[2].tensor_copy(a[0][:, off:], t[:, off:])
                                a[1] = off
                            else:
                                a[2].tensor_add(a[0][:, off:],
                                                a[0][:, off:], t[:, off:])
                        ess = [None] * n_kb
                        offs = [max(0, kb - 4 * qb) * 128
                                for kb in range(n_kb)]

                        def drain(kb, ot_ps=ot_ps, ess=ess, n_kb=n_kb,
                                  offs=offs):
                            o = offs[kb]
                            nc.tensor.matmul(
                                ot_ps[:, o:], vN[:, kb, :], ess[kb][:, o:],
                                start=(kb == 0), stop=(kb == n_kb - 1),
                            )

                        for kb in range(n_kb):
                            off = offs[kb]
                            s_ps = psS.tile([128, 512], F32, tag="sps",
                                            name="sps")
                            nc.tensor.matmul(
                                s_ps[:, off:],
                                kT[:, kb * 128:(kb + 1) * 128],
                                qTs[h][:, qb * 512 + off:(qb + 1) * 512],
                                start=True, stop=True,
                            )
                            es = expp.tile([128, 512], BF16, tag="es",
                                           name="es")
                            nc.scalar.activation(
                                es[:, off:], s_ps[:, off:],
                                mybir.ActivationFunctionType.Exp,
                                scale=SCALE,
                            )
                            if kb - 4 * qb >= 0:
                                # in-place 0/1 lower-triangle mask on the
                                # 128-wide diagonal sub-block
                                nc.vector.tensor_mul(
                                    es[:, off:off + 128],
                                    es[:, off:off + 128], tri_sb,
                                )
                            ess[kb] = es
                            # row-sum: full-width blocks reduce pairwise in
                            # bf16 on the DVE 2x path first (error averages
                            # out over the pair), then the f32 accumulators
                            # alternate GPSIMD/DVE
                            if off == 0 and kb < 4 * qb:
                                if pend[0] is None:
                                    pend[0] = es
                                else:
                                    esp = expp.tile([128, 512], BF16,
                                                    tag="es", name="esp")
                                    nc.vector.tensor_add(esp, pend[0], es)
                                    pend[0] = None
                                    acc_push(esp, 0)
                            else:
                                acc_push(es, off)
                            if kb >= LAG:
                                drain(kb - LAG)
                            # previous head's deferred chain, then this
                            # head's share of the qb-1 projection work
                            if kb == 0:
                                flush(2)
                            elif kb == 1:
                                flush(2)
                            elif kb == 2:
                                flush()
                            elif citems:
                                want = (len(citems) * (kb - 2)) // (n_kb - 3)
                                while c_done < want:
                                    citems[c_done]()
                                    c_done += 1
                        while c_done < len(citems):
                            citems[c_done]()
                            c_done += 1
                        for kb in range(max(0, n_kb - LAG), n_kb):
                            deferred.append(
                                lambda kb=kb, drain=drain: drain(kb)
                            )

                        def dchain(qb=qb, h=h, ot_ps=ot_ps, accs=accs):
                            # D broadcast across partitions (each output row
                            # of ones^T @ es_sum is the key-dim column sum),
                            # fast reciprocal, O^T scale
                            d_ps = psD.tile([128, 512], F32, tag="dps",
                                            name="dps")
                            live = sorted((a for a in accs
                                           if a[1] is not None),
                                          key=lambda a: a[1])
                            for i, (t, o, _) in enumerate(live):
                                nc.tensor.matmul(
                                    d_ps[:, o:], ones_sb, t[:, o:],
                                    start=(i == 0), stop=(i == len(live) - 1),
                                )
                            rd = rdp.tile([128, 512], F32, tag="rd",
                                          name="rd")
                            nc.vector.reciprocal_approx_fast(out=rd, in_=d_ps)
                            ot = otp.tile([128, 512], BF16, tag=f"ot{h}",
                                          name=f"ot{h}")
                            nc.vector.tensor_mul(ot, ot_ps, rd)
                            ot_store[(qb, h)] = ot

                        deferred.append(dchain)

                # trailing projection of the last query block
                flush()
                for qc in range(QH):
                    for it in c_items(NSB - 1, qc, last=True):
                        it()
    nc.finalize()
    return nc


_NC_CACHE = {}


def _get_nc():
    if "nc" not in _NC_CACHE:
        _NC_CACHE["nc"] = build_nc()
    return _NC_CACHE["nc"]


def _host_prep(x, cos, sin, mask, wq, wk, wv, wo):
    import ml_dtypes

    bf16 = ml_dtypes.bfloat16
    # partition-major shuffles: index [p, ...] with contraction tile t so
    # every DMA line is 4-8 KiB contiguous
    xS = np.ascontiguousarray(
        x[0].astype(bf16)                    # (S, D) = (sb*512+s, t*128+p)
        .reshape(NSB, 512, NKT, 128)
        .transpose(3, 0, 2, 1)               # (p, sb, t, s)
    )
    cosT = np.ascontiguousarray(cos[:, 0, :].T).astype(bf16)
    sinT = sin[:, 0, :].T.astype(np.float32)
    sinTs = np.ascontiguousarray(
        np.concatenate([-sinT[:64], sinT[64:]], axis=0)
    ).astype(bf16)
    rr = np.arange(128, dtype=np.int64)[:, None]
    cc = np.arange(128, dtype=np.int64)[None, :]
    tri = (rr <= cc).astype(np.float32).astype(bf16)
    ident = np.eye(128).astype(bf16)
    ones_mat = np.ones((128, 128), dtype=np.float32)

    def wshuf(w):
        # (t*128+p, m) -> (p, t*M+m)
        t = w.shape[0] // 128
        return np.ascontiguousarray(
            w.astype(bf16).reshape(t, 128, -1).transpose(1, 0, 2)
            .reshape(128, -1)
        )

    in_maps = []
    for i in range(N_CORES):
        in_maps.append({
            "xS": xS,
            "wqS": wshuf(wq[:, i * QS:(i + 1) * QS]),
            "wkS": wshuf(wk[:, i * 128:(i + 1) * 128]),
            "wvS": wshuf(wv[:, i * 128:(i + 1) * 128]),
            "woS": wshuf(wo[i * QS:(i + 1) * QS, :]),
            "cosT": cosT,
            "sinTs": sinTs,
            "tri": tri,
            "ident": ident,
            "ones_mat": ones_mat,
        })
    return in_maps


def kernel(x, cos, sin, mask, wq, wk, wv, wo, _trace=False, _trace_kwargs=None):
    nc = _get_nc()
    in_maps = _host_prep(x, cos, sin, mask, wq, wk, wv, wo)
    res = run_bass_kernel_spmd(
        nc, in_maps, list(range(N_CORES)), trace=_trace,
        **(_trace_kwargs or {}),
    )
    partials = [res.results[i]["out"] for i in range(N_CORES)]
    full = np.sum(
        np.stack([p.astype(np.float32) for p in partials], axis=0),
        axis=0, dtype=np.float64,
    )
    out = full.astype(np.float32)[None, :, :]
    if _trace:
        return out, res
    return out
